# revision 29
# baseline (speedup 1.0000x reference)
"""Causal self-attention (B=2, T=4096, C=768, H=12) on 8 trn2 NeuronCores — v3.

Sharding: core c -> batch b = c//4, head group g = c%4 (3 heads per core).

v3 changes vs v2 (319us):
  - All heads self-paired: each S step computes k-tile pair (j0,j1) of ONE
    head concurrently via tile_position (0,0)/(64,0). The partition-duplicated
    Q^T/K^T come from one SBUF->SBUF "swap halves" DMA per qkv m-group block
    (QTa=[q0|q1] natural psum copy, QTb=swap(QTa)=[q1|q0]) instead of
    duplicated weight columns -> QKV shrinks to 3 m-groups.
  - PV in fp8e4 with perf_mode=DoubleRow: one MM contracts both k-tiles of a
    step (V' [128,2,128] stationary, pt [128,2,512] moving) -> halves PV
    stream time. Diagonal steps stay split (2 plain fp8 MMs) to skip the
    invalid above-diagonal columns.
  - exp split across ScalarE and VectorE: ScalarE = ACTIVATE Exp (fp8 out);
    VectorE = single tensor_scalar op computing round(s*8*log2e + 55.55) into
    int8 = the fp8e4m3 BIT PATTERN of exp(s) (Schraudolph in fp8 bits; f32->
    int8 convert is round-to-nearest + saturating on HW, so -30000-masked
    scores land at -128 = -0.0 fp8). Blocks i=0 keep an exact bf16 path
    (ACT exp -> bf16 pt, bf16 V) because early rows have tiny L_eff; i=1 uses
    ACT fp8; i>=2 steps route by a fractional accumulator to balance engines.
  - Causal masking via PE: diagonal tiles get M_tri (upper=-30000) added in
    PSUM by an identity-weight matmul appended to the S accumulation group;
    no DVE mask multiplies remain.
  - l (softmax denom) via V' col of 8.0s: h1/h2 layouts [8|V] put l*8 at psum
    partition 0 -> direct DVE reciprocal_approx_fast from PSUM; h0 keeps
    [V|8] + partition-0 hop (DMA) + deferred norm. V scaled x8 into fp8 to
    dodge e4m3 subnormals; the 8s cancel in Y*(1/(8l))*8.
  - q/k psum->SBUF copies on ScalarE (Identity + per-partition bias), proj
    psum->SBUF on ScalarE (Copy, bf16 out); output DMA'd as bf16.
"""

import os
import sys

import numpy as np

for _p in ("/opt/trn_rl_repo", "/root/.axon_site/_ro/trn_rl_repo"):
    if os.path.isdir(_p) and _p not in sys.path:
        sys.path.insert(0, _p)

import ml_dtypes

import concourse.bacc as bacc
import concourse.bass as bass
import concourse.mybir as mybir
import concourse.tile as tile
from concourse.bass_utils import run_bass_kernel_spmd

B, T, C = 2, 4096, 768
H, HD = 12, 64
NCORES = 8
HPC = 3
P = 128
NBLK = T // 512
NKT = T // 128
NPAIR = NKT // 2
KC = C // 128

F32 = mybir.dt.float32
BF16 = mybir.dt.bfloat16
FP8 = mybir.dt.float8e4
I8 = mybir.dt.int8
BF16_NP = ml_dtypes.bfloat16
FP8_NP = ml_dtypes.float8_e4m3fn
AF = mybir.ActivationFunctionType
ALU = mybir.AluOpType
DR = mybir.MatmulPerfMode.DoubleRow

LOG2E = 1.4426950408889634
A8 = 8.0 * LOG2E
B8 = 55.55

LAG = 5          # PV trails exp by LAG steps
DVE_SHARE = 0.39  # fraction of i>=1 off-diag exp steps routed to VectorE
                  # (all diagonal steps are forced to VectorE via MB8)

MASKVAL = -30000.0

_CACHE = {}


def _build_nc():
    nc = bacc.Bacc("TRN2", target_bir_lowering=False, debug=False)

    xt_d = nc.dram_tensor("xt", [C, T], BF16, kind="ExternalInput")
    wqk_d = nc.dram_tensor("wqk", [C, 3 * P], BF16, kind="ExternalInput")
    wv_d = nc.dram_tensor("wv", [C, HPC * HD], BF16, kind="ExternalInput")
    wp_d = nc.dram_tensor("wp", [P, 2, C], BF16, kind="ExternalInput")
    bias_d = nc.dram_tensor("bias_qk", [P, 3], F32, kind="ExternalInput")
    bv_d = nc.dram_tensor("bv8", [P, HPC * HD], F32, kind="ExternalInput")
    mtri_d = nc.dram_tensor("mtri", [P, P], BF16, kind="ExternalInput")
    idm_d = nc.dram_tensor("idm", [P, P], BF16, kind="ExternalInput")
    mb8_d = nc.dram_tensor("mb8", [P, 2, 1024], BF16, kind="ExternalInput")
    out_d = nc.dram_tensor("outT", [C, T], BF16, kind="ExternalOutput")

    with tile.TileContext(nc) as tc:
        with (
            tc.tile_pool(name="store", bufs=1) as store,
            tc.tile_pool(name="consts", bufs=1) as consts,
            tc.tile_pool(name="pt8_pool", bufs=10) as pt8_pool,
            tc.tile_pool(name="ptb_pool", bufs=3) as ptb_pool,
            tc.tile_pool(name="rsb_pool", bufs=2) as rsb_pool,
            tc.tile_pool(name="rb_pool", bufs=2) as rb_pool,
            tc.tile_pool(name="osb_pool", bufs=3) as osb_pool,
            tc.tile_pool(name="s_psum", bufs=1, space="PSUM") as s_psum,
            tc.tile_pool(name="y_psum", bufs=1, space="PSUM") as y_psum,
            tc.tile_pool(name="m_psum", bufs=1, space="PSUM") as m_psum,
        ):
            # ---- persistent SBUF ----
            XT = store.tile([P, KC, T], BF16)
            WQK = store.tile([P, KC, 3 * P], BF16)
            WV = store.tile([P, KC, HPC * HD], BF16)
            WP = store.tile([P, 2, C], BF16)
            QTa = store.tile([P, T], BF16)
            QTb = store.tile([P, T], BF16)
            KTa = store.tile([P, T], BF16)
            KTb = store.tile([P, T], BF16)
            QKa = store.tile([P, T], BF16)
            QKb = store.tile([P, T], BF16)
            # V' per (pair c, head h, slice s): M-layout
            #   h0: [V(0:64) | 8@64 | 0...]  (Y at psum 0-63, l*8 at 64)
            #   h1/h2: [8@0 | 0 | V(64:128)] (l*8 at psum 0, Y at 64-127)
            VN = store.tile([P, NPAIR, HPC, 2, P], FP8)
            VNB = store.tile([P, 2, HPC, 2, P], BF16)  # pairs 0-1, bf16 for i=0
            YN01 = store.tile([P, T], BF16)  # h0 rows 0-63, h1 rows 64-127
            YN2 = store.tile([P, T], BF16)   # h2 rows 64-127

            bias_qk = consts.tile([P, 3], F32)
            bv8 = consts.tile([P, HPC * HD], F32)
            MTRI = consts.tile([P, P], BF16)
            IDM = consts.tile([P, P], BF16)
            MB8 = consts.tile([P, 2, 1024], BF16)

            # ---- input DMAs: block-0 critical path first ----
            nc.sync.dma_start(WQK[:], wqk_d.rearrange("(k p) c -> p k c", p=P))
            nc.sync.dma_start(bias_qk[:], bias_d[:])
            xt_view = xt_d.rearrange("(k p) t -> p k t", p=P)
            for k in range(KC):
                nc.sync.dma_start(XT[:, k, 0:512], xt_view[:, k, 0:512])
            nc.sync.dma_start(WV[:], wv_d.rearrange("(k p) c -> p k c", p=P))
            nc.sync.dma_start(bv8[:], bv_d[:])
            nc.sync.dma_start(MTRI[:], mtri_d[:])
            nc.sync.dma_start(IDM[:], idm_d[:])
            nc.sync.dma_start(MB8[:], mb8_d[:])
            nc.sync.dma_start(WP[:], wp_d[:])
            for n in range(1, NBLK):
                nc.gpsimd.dma_start(
                    XT[:, :, n * 512 : (n + 1) * 512],
                    xt_view[:, :, n * 512 : (n + 1) * 512],
                )

            nc.any.memset(VN[:], 0.0)
            nc.any.memset(VN[:, :, 0, :, HD : HD + 1], 8.0)
            nc.any.memset(VN[:, :, 1, :, 0:1], 8.0)
            nc.any.memset(VN[:, :, 2, :, 0:1], 8.0)
            nc.any.memset(VNB[:], 0.0)
            nc.any.memset(VNB[:, :, 0, :, HD : HD + 1], 8.0)
            nc.any.memset(VNB[:, :, 1, :, 0:1], 8.0)
            nc.any.memset(VNB[:, :, 2, :, 0:1], 8.0)

            # ---- qkv / v / proj groups ----
            DSTA = (QTa, KTa, QKa)
            DSTB = (QTb, KTb, QKb)

            def misc_tile(alt):
                if alt:
                    mt_y = y_psum.tile([P, 512], F32, tag="y0")
                    return mt_y
                mt_m = m_psum.tile([P, 512], F32, tag="misc")
                return mt_m

            # fillers are split into an MM part and a copy part so the
            # ACT/DVE copy never sits at an engine-queue head waiting on
            # its own matmul (strict per-engine FIFO would stall the exps
            # queued behind it).
            def qkv_mms(m, n, alt=False):
                ps = misc_tile(alt)
                for k in range(KC):
                    nc.tensor.matmul(
                        ps[:],
                        WQK[:, k, m * P : (m + 1) * P],
                        XT[:, k, n * 512 : (n + 1) * 512],
                        start=(k == 0),
                        stop=(k == KC - 1),
                    )
                return ps

            def qkv_copy(ps, m, n):
                blk = slice(n * 512, (n + 1) * 512)
                da, db = DSTA[m], DSTB[m]
                nc.scalar.activation(
                    da[:, blk], ps[:], AF.Identity, bias=bias_qk[:, m : m + 1]
                )
                nc.sync.dma_start(db[0:HD, blk], da[HD:P, blk])
                nc.sync.dma_start(db[HD:P, blk], da[0:HD, blk])

            def v_mms(mt, alt=False):
                pst = misc_tile(alt)
                for k in range(KC):
                    nc.tensor.matmul(
                        pst[:, 0 : HPC * HD],
                        XT[:, k, mt * P : (mt + 1) * P],
                        WV[:, k, :],
                        start=(k == 0),
                        stop=(k == KC - 1),
                    )
                return pst

            def v_copy(pst, mt):
                c, s = mt // 2, mt % 2
                vpv = pst[:, 0 : HPC * HD].rearrange("p (h d) -> p h d", h=HPC)
                bvv = bv8[:].rearrange("p (h d) -> p h d", h=HPC)
                nc.vector.scalar_tensor_tensor(
                    VN[:, c, 0, s, 0:HD], pst[:, 0:HD], 8.0, bv8[:, 0:HD],
                    op0=ALU.mult, op1=ALU.add,
                )
                nc.vector.scalar_tensor_tensor(
                    VN[:, c, 1:3, s, HD:P], vpv[:, 1:3, :], 8.0, bvv[:, 1:3, :],
                    op0=ALU.mult, op1=ALU.add,
                )
                if mt < 4:
                    nc.vector.scalar_tensor_tensor(
                        VNB[:, c, 0, s, 0:HD], pst[:, 0:HD], 8.0, bv8[:, 0:HD],
                        op0=ALU.mult, op1=ALU.add,
                    )
                    nc.vector.scalar_tensor_tensor(
                        VNB[:, c, 1:3, s, HD:P], vpv[:, 1:3, :], 8.0,
                        bvv[:, 1:3, :], op0=ALU.mult, op1=ALU.add,
                    )

            def proj_mms(m, n, alt=False):
                ops = misc_tile(alt)
                nc.tensor.matmul(
                    ops[:],
                    WP[:, 0, m * P : (m + 1) * P],
                    YN01[:, n * 512 : (n + 1) * 512],
                    start=True,
                    stop=False,
                )
                nc.tensor.matmul(
                    ops[:],
                    WP[HD:P, 1, m * P : (m + 1) * P],
                    YN2[HD:P, n * 512 : (n + 1) * 512],
                    start=False,
                    stop=True,
                    tile_position=(HD, 0),
                )
                return ops

            def proj_copy(ops, m, n):
                osb = osb_pool.tile([P, 512], BF16)
                nc.scalar.activation(osb[:], ops[:], AF.Copy)
                nc.sync.dma_start(
                    out_d[m * P : (m + 1) * P, n * 512 : (n + 1) * 512],
                    osb[:],
                )

            def make_split(mm_fn, copy_fn):
                cell = {}

                def p1(alt=False):
                    cell["ps"] = mm_fn(alt)

                def p2(alt=False):
                    copy_fn(cell["ps"])

                return p1, p2

            # ---- filler queue ----
            from collections import deque

            filler_q = deque()
            chunk_done = [0]

            def pop_filler(k):
                for _ in range(k):
                    if not filler_q:
                        return
                    n_final, fn = filler_q.popleft()
                    fn()
                    if n_final is not None:
                        chunk_done[0] = max(chunk_done[0], n_final)

            def drain_through_chunk(n):
                while filler_q and chunk_done[0] < n:
                    pop_filler(1)

            deferred = []
            deferred2 = []

            def flush_norms():
                while deferred:
                    deferred.pop(0)()

            def flush_norms2():
                while deferred2:
                    deferred2.pop(0)()

            # exp routing accumulator
            route_acc = [0.0]

            def route_dve():
                route_acc[0] += DVE_SHARE
                if route_acc[0] >= 1.0:
                    route_acc[0] -= 1.0
                    return True
                return False

            # S operand tables per head: (KT_lo, KT_hi, QT_lo, QT_hi)
            SOPS = (
                (KTa, KTb, QTa, QTb),
                (KTb, KTa, QTb, QTa),
                (QKb, QKa, QKa, QKb),
            )

            gstep = [0]

            def attn_block(i, h):
                accurate = i == 0
                kt_lo, kt_hi, qt_lo, qt_hi = SOPS[h]
                yps = y_psum.tile([P, 512], F32, tag="y0")
                vsrc = VNB if accurate else VN
                clast = 2 * i + 1
                pending = []

                def emit_pv(ent):
                    pt, c, off0, off1, first, last = ent
                    if not accurate:
                        nc.tensor.matmul(
                            yps[:],
                            vsrc[:, c, h, :, :],
                            pt[:].rearrange("p (s n) -> p s n", s=2),
                            start=first,
                            stop=last,
                            perf_mode=DR,
                        )
                    else:
                        nc.tensor.matmul(
                            yps[:, off0:],
                            vsrc[:, c, h, 0, :],
                            pt[:, off0:512],
                            start=first,
                            stop=False,
                        )
                        nc.tensor.matmul(
                            yps[:, off1:],
                            vsrc[:, c, h, 1, :],
                            pt[:, 512 + off1 : 1024],
                            start=False,
                            stop=last,
                        )

                # separate the two DVE-forced diagonal steps: first and last
                if accurate:
                    order = list(range(2 * i + 2))
                else:
                    order = [2 * i] + list(range(2 * i)) + [2 * i + 1]
                nlast = len(order) - 1
                f1 = min(3, nlast)
                f2 = min(5, nlast)
                for ci, c in enumerate(order):
                    first = ci == 0
                    last = ci == nlast
                    j0, j1 = 2 * c, 2 * c + 1
                    off0 = max(0, j0 - 4 * i) * P
                    off1 = max(0, j1 - 4 * i) * P
                    diag = j1 >= 4 * i
                    # ready work FIRST so it sits ahead of the dependent S
                    # matmuls in the engine FIFOs
                    if ci == f1:
                        flush_norms()
                    if ci == f2:
                        flush_norms2()
                    if len(pending) > LAG:
                        emit_pv(pending.pop(0))
                    if i < 3:
                        if ci % 2 == 1:
                            pop_filler(1)
                    elif ci % 3 == 2:
                        pop_filler(1)
                    sps = s_psum.tile(
                        [P, 1024], F32, tag=f"s{gstep[0] % 3}"
                    )
                    gstep[0] += 1
                    tri0 = accurate and j0 >= 4 * i
                    tri1 = accurate and j1 >= 4 * i
                    nc.tensor.matmul(
                        sps[:, off0:512],
                        kt_lo[0:HD, j0 * P : (j0 + 1) * P],
                        qt_lo[0:HD, i * 512 + off0 : (i + 1) * 512],
                        start=True,
                        stop=not tri0,
                        tile_position=(0, 0),
                    )
                    nc.tensor.matmul(
                        sps[:, 512 + off1 : 1024],
                        kt_hi[HD:P, j1 * P : (j1 + 1) * P],
                        qt_hi[HD:P, i * 512 + off1 : (i + 1) * 512],
                        start=True,
                        stop=not tri1,
                        tile_position=(HD, 0),
                    )
                    if tri0:
                        nc.tensor.matmul(
                            sps[:, off0 : off0 + P],
                            IDM[:],
                            MTRI[:],
                            start=False,
                            stop=True,
                            skip_group_check=True,
                        )
                    if tri1:
                        nc.tensor.matmul(
                            sps[:, 512 + off1 : 512 + off1 + P],
                            IDM[:],
                            MTRI[:],
                            start=False,
                            stop=True,
                            skip_group_check=True,
                        )
                    if accurate:
                        pt = ptb_pool.tile([P, 1024], BF16, tag="ptb")
                        nc.scalar.activation(pt[:, off0:], sps[:, off0:], AF.Exp)
                    elif diag:
                        # DVE exp with mask-bias tensor: (s*A8) + MB8 -> int8
                        # bits of fp8 exp; masked/stale lanes saturate to -0.0
                        pt = pt8_pool.tile([P, 1024], FP8, tag="pt8")
                        nc.vector.scalar_tensor_tensor(
                            pt[:].bitcast(I8),
                            sps[:],
                            A8,
                            MB8[:, c % 2, :],
                            op0=ALU.mult,
                            op1=ALU.add,
                        )
                    elif not route_dve():
                        pt = pt8_pool.tile([P, 1024], FP8, tag="pt8")
                        nc.scalar.activation(pt[:], sps[:], AF.Exp)
                    else:
                        pt = pt8_pool.tile([P, 1024], FP8, tag="pt8")
                        nc.vector.tensor_scalar(
                            pt[:].bitcast(I8),
                            sps[:],
                            A8,
                            B8,
                            op0=ALU.mult,
                            op1=ALU.add,
                        )
                    pending.append((pt, c, off0, off1, first, last))
                while pending:
                    emit_pv(pending.pop(0))

                # ---- normalize (fully deferred into the next head's steps
                # so recip/mul never head-block the DVE exp queue) ----
                blk = slice(i * 512, (i + 1) * 512)
                if h == 0:
                    # Y*8 at psum 0-63, l*8 at partition 64: needs partition
                    # hop; two-stage deferral (copy+hop, then recip+mul)
                    ls = rsb_pool.tile([P, 512], F32, tag="ls0", bufs=1)
                    lr = rsb_pool.tile([1, 512], F32, tag="lr0", bufs=1)
                    r0 = rsb_pool.tile([P, 512], F32, tag="r0", bufs=1)
                    rb0 = rb_pool.tile([P, 512], F32, tag="rb0", bufs=1)

                    def _hop_h0(yps=yps, ls=ls, lr=lr):
                        nc.scalar.activation(
                            ls[HD : HD + 1, :], yps[HD : HD + 1, :], AF.Copy
                        )
                        nc.gpsimd.dma_start(lr[0:1, :], ls[HD : HD + 1, :])

                    def _norm_h0(i=i, yps=yps, lr=lr, r0=r0, rb0=rb0, blk=blk):
                        nc.vector.reciprocal_approx_fast(r0[0:1, :], lr[0:1, :])
                        nc.gpsimd.partition_broadcast(rb0[:, :], r0[0:1, :])
                        nc.vector.tensor_mul(
                            YN01[0:HD, blk], yps[0:HD, :], rb0[0:HD, :]
                        )

                    deferred.append(_hop_h0)
                    deferred2.append(_norm_h0)
                else:
                    # l*8 at psum partition 0: single-stage deferral
                    rtag = "r1" if h == 1 else "r2"
                    r1 = rsb_pool.tile([P, 512], F32, tag=rtag, bufs=1)
                    rb1 = rb_pool.tile([P, 512], F32, tag="rb" + rtag, bufs=1)
                    dst = YN01 if h == 1 else YN2

                    def _norm_h(yps=yps, r1=r1, rb1=rb1, dst=dst, blk=blk):
                        nc.vector.reciprocal_approx_fast(r1[:, :], yps[:, :])
                        nc.gpsimd.partition_broadcast(rb1[:, :], r1[0:1, :])
                        nc.vector.tensor_mul(
                            dst[HD:P, blk], yps[HD:P, :], rb1[HD:P, :]
                        )

                    deferred.append(_norm_h)

            # ---- prologue: block-0 qkv/v dense (alternate psum banks) ----
            for m in range(3):
                ps = qkv_mms(m, 0, alt=(m % 2 == 1))
                qkv_copy(ps, m, 0)
            for mt in range(4):
                pst = v_mms(mt, alt=(mt % 2 == 0))
                v_copy(pst, mt)

            for n in range(1, NBLK):
                for m in range(3):
                    p1, p2 = make_split(
                        (lambda alt, m=m, n=n: qkv_mms(m, n, alt)),
                        (lambda ps, m=m, n=n: qkv_copy(ps, m, n)),
                    )
                    filler_q.append((None, p1))
                    filler_q.append((None, p2))
                for s in range(4):
                    p1, p2 = make_split(
                        (lambda alt, t=4 * n + s: v_mms(t, alt)),
                        (lambda ps, t=4 * n + s: v_copy(ps, t)),
                    )
                    filler_q.append((None, p1))
                    filler_q.append((n if s == 3 else None, p2))

            def append_proj(n):
                for m in range(KC):
                    p1, p2 = make_split(
                        (lambda alt, m=m, n=n: proj_mms(m, n, alt)),
                        (lambda ps, m=m, n=n: proj_copy(ps, m, n)),
                    )
                    filler_q.append((None, p1))
                    filler_q.append((None, p2))

            # ---- main pipeline ----
            for i in range(NBLK):
                drain_through_chunk(i)
                for h in range(HPC):
                    attn_block(i, h)
                    pop_filler(1)
                    # proj for block i-1: only after h0(i)'s in-loop flush has
                    # emitted ALL of block i-1's norm writes (h2 flushes here)
                    if h == 0 and i > 0:
                        append_proj(i - 1)

            flush_norms()
            flush_norms2()
            append_proj(NBLK - 1)
            alt = False
            while filler_q:
                n_final, fn = filler_q.popleft()
                fn(alt=alt)
                alt = not alt

    nc.compile()
    return nc


def _per_core_inputs(c, x, w_attn, b_attn, w_proj, xt_cache):
    b, g = divmod(c, 4)
    hs = [HPC * g + j for j in range(HPC)]

    if b not in xt_cache:
        xt_cache[b] = np.ascontiguousarray(x[b].T).astype(BF16_NP)
    xt = xt_cache[b]

    sc = 1.0 / np.sqrt(np.float32(HD))
    qc = lambda h: w_attn[:, h * HD : (h + 1) * HD] * sc
    kc = lambda h: w_attn[:, C + h * HD : C + (h + 1) * HD]
    # m-groups: [q0|q1], [k0|k1], [q2|k2]
    wqk = np.concatenate(
        [qc(hs[0]), qc(hs[1]), kc(hs[0]), kc(hs[1]), qc(hs[2]), kc(hs[2])],
        axis=1,
    ).astype(BF16_NP)
    wv = np.concatenate(
        [w_attn[:, 2 * C + h * HD : 2 * C + (h + 1) * HD] for h in hs], axis=1
    ).astype(BF16_NP)

    bq = lambda h: b_attn[h * HD : (h + 1) * HD] * sc
    bk = lambda h: b_attn[C + h * HD : C + (h + 1) * HD]
    bias_qk = np.stack(
        [
            np.concatenate([bq(hs[0]), bq(hs[1])]),
            np.concatenate([bk(hs[0]), bk(hs[1])]),
            np.concatenate([bq(hs[2]), bk(hs[2])]),
        ],
        axis=1,
    ).astype(np.float32)
    bv8 = np.broadcast_to(
        8.0
        * np.concatenate(
            [b_attn[2 * C + h * HD : 2 * C + (h + 1) * HD] for h in hs]
        ).astype(np.float32)[None, :],
        (P, HPC * HD),
    ).copy()

    # wp: slot0 = [wp_h0; wp_h1]; slot1 rows 64-127 = wp_h2
    wp0 = np.concatenate(
        [
            w_proj[hs[0] * HD : (hs[0] + 1) * HD, :],
            w_proj[hs[1] * HD : (hs[1] + 1) * HD, :],
        ]
    )
    wp1 = np.concatenate(
        [
            np.zeros((HD, C), np.float32),
            w_proj[hs[2] * HD : (hs[2] + 1) * HD, :],
        ]
    )
    wp = np.stack([wp0, wp1], axis=1).astype(BF16_NP)

    kk = np.arange(P)[:, None]
    qq = np.arange(P)[None, :]
    mtri = np.where(kk <= qq, 0.0, MASKVAL).astype(BF16_NP)
    idm = np.eye(P, dtype=BF16_NP)

    # MB8: per-element bias for the DVE fp8-bits exp on diagonal steps.
    # B8 where the score is valid, -1e6 where masked or stale.
    BIG = -1.0e6
    tri = np.where(kk <= qq, B8, BIG).astype(np.float32)  # [k, q] in-tile
    mb8 = np.empty((2, P, 1024), np.float32)
    # pattern 0: step c=2i  (j0 diag at subtile 0, j1 at subtile 1)
    mb8[0, :, :] = B8
    mb8[0, :, 0:128] = tri
    mb8[0, :, 512:640] = BIG
    mb8[0, :, 640:768] = tri
    # pattern 1: step c=2i+1 (j0 diag at subtile 2, j1 at subtile 3)
    mb8[1, :, :] = B8
    mb8[1, :, 0:256] = BIG
    mb8[1, :, 256:384] = tri
    mb8[1, :, 512:896] = BIG
    mb8[1, :, 896:1024] = tri
    mb8 = np.ascontiguousarray(mb8.transpose(1, 0, 2)).astype(BF16_NP)

    return {
        "xt": xt,
        "wqk": wqk,
        "wv": wv,
        "wp": wp,
        "bias_qk": bias_qk,
        "bv8": bv8,
        "mtri": mtri,
        "idm": idm,
        "mb8": mb8,
    }


def build_in_maps(x, w_attn, b_attn, w_proj):
    x = np.asarray(x, np.float32)
    w_attn = np.asarray(w_attn, np.float32)
    b_attn = np.asarray(b_attn, np.float32)
    w_proj = np.asarray(w_proj, np.float32)

    xt_cache = {}
    return [
        _per_core_inputs(c, x, w_attn, b_attn, w_proj, xt_cache)
        for c in range(NCORES)
    ]


def kernel(x, w_attn, b_attn, w_proj, b_proj, _return_raw=False):
    x = np.asarray(x, np.float32)
    b_proj = np.asarray(b_proj, np.float32)

    if "nc" not in _CACHE:
        _CACHE["nc"] = _build_nc()
    nc = _CACHE["nc"]

    in_maps = build_in_maps(x, w_attn, b_attn, w_proj)
    res = run_bass_kernel_spmd(nc, in_maps, list(range(NCORES)))
    outs = [r["outT"] for r in res.results]

    full = np.empty((B, T, C), np.float32)
    for b in range(B):
        acc = outs[4 * b].astype(np.float32)
        for g in range(1, 4):
            acc += outs[4 * b + g].astype(np.float32)
        full[b] = acc.T
    full += b_proj[None, None, :]
    if _return_raw:
        return full, res
    return full


# revision 33
# speedup vs baseline: 1.2546x; 1.2546x over previous
"""Causal self-attention (B=2, T=4096, C=768, H=12) on 8 trn2 NeuronCores — v3.

Sharding: core c -> batch b = c//4, head group g = c%4 (3 heads per core).

v3 changes vs v2 (319us):
  - All heads self-paired: each S step computes k-tile pair (j0,j1) of ONE
    head concurrently via tile_position (0,0)/(64,0). The partition-duplicated
    Q^T/K^T come from one SBUF->SBUF "swap halves" DMA per qkv m-group block
    (QTa=[q0|q1] natural psum copy, QTb=swap(QTa)=[q1|q0]) instead of
    duplicated weight columns -> QKV shrinks to 3 m-groups.
  - PV in fp8e4 with perf_mode=DoubleRow: one MM contracts both k-tiles of a
    step (V' [128,2,128] stationary, pt [128,2,512] moving) -> halves PV
    stream time. Diagonal steps stay split (2 plain fp8 MMs) to skip the
    invalid above-diagonal columns.
  - exp split across ScalarE and VectorE: ScalarE = ACTIVATE Exp (fp8 out);
    VectorE = single tensor_scalar op computing round(s*8*log2e + 55.55) into
    int8 = the fp8e4m3 BIT PATTERN of exp(s) (Schraudolph in fp8 bits; f32->
    int8 convert is round-to-nearest + saturating on HW, so -30000-masked
    scores land at -128 = -0.0 fp8). Blocks i=0 keep an exact bf16 path
    (ACT exp -> bf16 pt, bf16 V) because early rows have tiny L_eff; i=1 uses
    ACT fp8; i>=2 steps route by a fractional accumulator to balance engines.
  - Causal masking via PE: diagonal tiles get M_tri (upper=-30000) added in
    PSUM by an identity-weight matmul appended to the S accumulation group;
    no DVE mask multiplies remain.
  - l (softmax denom) via V' col of 8.0s: h1/h2 layouts [8|V] put l*8 at psum
    partition 0 -> direct DVE reciprocal_approx_fast from PSUM; h0 keeps
    [V|8] + partition-0 hop (DMA) + deferred norm. V scaled x8 into fp8 to
    dodge e4m3 subnormals; the 8s cancel in Y*(1/(8l))*8.
  - q/k psum->SBUF copies on ScalarE (Identity + per-partition bias), proj
    psum->SBUF on ScalarE (Copy, bf16 out); output DMA'd as bf16.
"""

import os
import sys

import numpy as np

for _p in ("/opt/trn_rl_repo", "/root/.axon_site/_ro/trn_rl_repo"):
    if os.path.isdir(_p) and _p not in sys.path:
        sys.path.insert(0, _p)

import ml_dtypes

import concourse.bacc as bacc
import concourse.bass as bass
import concourse.mybir as mybir
import concourse.tile as tile
from concourse.bass_utils import run_bass_kernel_spmd

B, T, C = 2, 4096, 768
H, HD = 12, 64
NCORES = 8
HPC = 3
P = 128
NBLK = T // 512
NKT = T // 128
NPAIR = NKT // 2
KC = C // 128

F32 = mybir.dt.float32
BF16 = mybir.dt.bfloat16
FP8 = mybir.dt.float8e4
I8 = mybir.dt.int8
BF16_NP = ml_dtypes.bfloat16
FP8_NP = ml_dtypes.float8_e4m3fn
AF = mybir.ActivationFunctionType
ALU = mybir.AluOpType
DR = mybir.MatmulPerfMode.DoubleRow

LOG2E = 1.4426950408889634
A8 = 8.0 * LOG2E
B8 = 55.55

LAG = 5          # PV trails exp by LAG steps
DVE_SHARE = 0.39  # fraction of i>=1 off-diag exp steps routed to VectorE
                  # (all diagonal steps are forced to VectorE via MB8)

MASKVAL = -30000.0

_CACHE = {}


def _build_nc():
    nc = bacc.Bacc("TRN2", target_bir_lowering=False, debug=False)

    xt_d = nc.dram_tensor("xt", [C, T], BF16, kind="ExternalInput")
    wqk_d = nc.dram_tensor("wqk", [C, 3 * P], BF16, kind="ExternalInput")
    wv_d = nc.dram_tensor("wv", [C, HPC * HD], BF16, kind="ExternalInput")
    wp_d = nc.dram_tensor("wp", [P, 2, C], BF16, kind="ExternalInput")
    bias_d = nc.dram_tensor("bias_qk", [P, 3], F32, kind="ExternalInput")
    bv_d = nc.dram_tensor("bv8", [P, HPC * HD], F32, kind="ExternalInput")
    mtri_d = nc.dram_tensor("mtri", [P, P], BF16, kind="ExternalInput")
    idm_d = nc.dram_tensor("idm", [P, P], BF16, kind="ExternalInput")
    mb8_d = nc.dram_tensor("mb8", [P, 2, 1024], BF16, kind="ExternalInput")
    out_d = nc.dram_tensor("outT", [C, T], BF16, kind="ExternalOutput")

    with tile.TileContext(nc) as tc:
        with (
            tc.tile_pool(name="store", bufs=1) as store,
            tc.tile_pool(name="consts", bufs=1) as consts,
            tc.tile_pool(name="pt8_pool", bufs=10) as pt8_pool,
            tc.tile_pool(name="ptb_pool", bufs=3) as ptb_pool,
            tc.tile_pool(name="rsb_pool", bufs=2) as rsb_pool,
            tc.tile_pool(name="yu_pool", bufs=3) as yu_pool,
            tc.tile_pool(name="rb_pool", bufs=2) as rb_pool,
            tc.tile_pool(name="osb_pool", bufs=3) as osb_pool,
            tc.tile_pool(name="s_psum", bufs=1, space="PSUM") as s_psum,
            tc.tile_pool(name="y_psum", bufs=1, space="PSUM") as y_psum,
            tc.tile_pool(name="m_psum", bufs=1, space="PSUM") as m_psum,
        ):
            # ---- persistent SBUF ----
            XT = store.tile([P, KC, T], BF16)
            WQK = store.tile([P, KC, 3 * P], BF16)
            WV = store.tile([P, KC, HPC * HD], BF16)
            WP = store.tile([P, 2, C], BF16)
            QTa = store.tile([P, T], BF16)
            QTb = store.tile([P, T], BF16)
            KTa = store.tile([P, T], BF16)
            KTb = store.tile([P, T], BF16)
            QKa = store.tile([P, T], BF16)
            QKb = store.tile([P, T], BF16)
            # V' per (pair c, head h, slice s): M-layout
            #   h0: [V(0:64) | 8@64 | 0...]  (Y at psum 0-63, l*8 at 64)
            #   h1/h2: [8@0 | 0 | V(64:128)] (l*8 at psum 0, Y at 64-127)
            VN = store.tile([P, NPAIR, HPC, 2, P], FP8)
            VNB = store.tile([P, 2, HPC, 2, P], BF16)  # pairs 0-1, bf16 for i=0
            YN01 = store.tile([P, T], BF16)  # h0 rows 0-63, h1 rows 64-127
            YN2 = store.tile([P, T], BF16)   # h2 rows 64-127

            bias_qk = consts.tile([P, 3], F32)
            bv8 = consts.tile([P, HPC * HD], F32)
            MTRI = consts.tile([P, P], BF16)
            IDM = consts.tile([P, P], BF16)
            MB8 = consts.tile([P, 2, 1024], BF16)

            # ---- input DMAs: block-0 critical path first ----
            nc.sync.dma_start(WQK[:], wqk_d.rearrange("(k p) c -> p k c", p=P))
            nc.sync.dma_start(bias_qk[:], bias_d[:])
            xt_view = xt_d.rearrange("(k p) t -> p k t", p=P)
            for k in range(KC):
                nc.sync.dma_start(XT[:, k, 0:512], xt_view[:, k, 0:512])
            nc.sync.dma_start(WV[:], wv_d.rearrange("(k p) c -> p k c", p=P))
            nc.sync.dma_start(bv8[:], bv_d[:])
            nc.sync.dma_start(MTRI[:], mtri_d[:])
            nc.sync.dma_start(IDM[:], idm_d[:])
            nc.sync.dma_start(MB8[:], mb8_d[:])
            nc.sync.dma_start(WP[:], wp_d[:])
            for n in range(1, NBLK):
                nc.gpsimd.dma_start(
                    XT[:, :, n * 512 : (n + 1) * 512],
                    xt_view[:, :, n * 512 : (n + 1) * 512],
                )

            nc.any.memset(VN[:], 0.0)
            nc.any.memset(VN[:, :, 0, :, HD : HD + 1], 8.0)
            nc.any.memset(VN[:, :, 1, :, 0:1], 8.0)
            nc.any.memset(VN[:, :, 2, :, 0:1], 8.0)
            nc.any.memset(VNB[:], 0.0)
            nc.any.memset(VNB[:, :, 0, :, HD : HD + 1], 8.0)
            nc.any.memset(VNB[:, :, 1, :, 0:1], 8.0)
            nc.any.memset(VNB[:, :, 2, :, 0:1], 8.0)

            # ---- qkv / v / proj groups ----
            DSTA = (QTa, KTa, QKa)
            DSTB = (QTb, KTb, QKb)

            def misc_tile(alt):
                if alt:
                    mt_y = y_psum.tile([P, 512], F32, tag="y0")
                    return mt_y
                mt_m = m_psum.tile([P, 512], F32, tag="misc")
                return mt_m

            # fillers are split into an MM part and a copy part so the
            # ACT/DVE copy never sits at an engine-queue head waiting on
            # its own matmul (strict per-engine FIFO would stall the exps
            # queued behind it).
            def qkv_mms(m, n, alt=False):
                ps = misc_tile(alt)
                for k in range(KC):
                    nc.tensor.matmul(
                        ps[:],
                        WQK[:, k, m * P : (m + 1) * P],
                        XT[:, k, n * 512 : (n + 1) * 512],
                        start=(k == 0),
                        stop=(k == KC - 1),
                    )
                return ps

            def qkv_copy(ps, m, n):
                blk = slice(n * 512, (n + 1) * 512)
                da, db = DSTA[m], DSTB[m]
                nc.scalar.activation(
                    da[:, blk], ps[:], AF.Identity, bias=bias_qk[:, m : m + 1]
                )
                nc.sync.dma_start(db[0:HD, blk], da[HD:P, blk])
                nc.sync.dma_start(db[HD:P, blk], da[0:HD, blk])

            def v_mms(mt, alt=False):
                pst = misc_tile(alt)
                for k in range(KC):
                    nc.tensor.matmul(
                        pst[:, 0 : HPC * HD],
                        XT[:, k, mt * P : (mt + 1) * P],
                        WV[:, k, :],
                        start=(k == 0),
                        stop=(k == KC - 1),
                    )
                return pst

            def v_copy(pst, mt):
                c, s = mt // 2, mt % 2
                vpv = pst[:, 0 : HPC * HD].rearrange("p (h d) -> p h d", h=HPC)
                bvv = bv8[:].rearrange("p (h d) -> p h d", h=HPC)
                nc.vector.scalar_tensor_tensor(
                    VN[:, c, 0, s, 0:HD], pst[:, 0:HD], 8.0, bv8[:, 0:HD],
                    op0=ALU.mult, op1=ALU.add,
                )
                nc.vector.scalar_tensor_tensor(
                    VN[:, c, 1:3, s, HD:P], vpv[:, 1:3, :], 8.0, bvv[:, 1:3, :],
                    op0=ALU.mult, op1=ALU.add,
                )
                if mt < 4:
                    nc.vector.scalar_tensor_tensor(
                        VNB[:, c, 0, s, 0:HD], pst[:, 0:HD], 8.0, bv8[:, 0:HD],
                        op0=ALU.mult, op1=ALU.add,
                    )
                    nc.vector.scalar_tensor_tensor(
                        VNB[:, c, 1:3, s, HD:P], vpv[:, 1:3, :], 8.0,
                        bvv[:, 1:3, :], op0=ALU.mult, op1=ALU.add,
                    )

            def proj_mms(m, n, alt=False):
                ops = misc_tile(alt)
                nc.tensor.matmul(
                    ops[:],
                    WP[:, 0, m * P : (m + 1) * P],
                    YN01[:, n * 512 : (n + 1) * 512],
                    start=True,
                    stop=False,
                )
                nc.tensor.matmul(
                    ops[:],
                    WP[HD:P, 1, m * P : (m + 1) * P],
                    YN2[HD:P, n * 512 : (n + 1) * 512],
                    start=False,
                    stop=True,
                    tile_position=(HD, 0),
                )
                return ops

            def proj_copy(ops, m, n):
                osb = osb_pool.tile([P, 512], BF16)
                nc.scalar.activation(osb[:], ops[:], AF.Copy)
                nc.sync.dma_start(
                    out_d[m * P : (m + 1) * P, n * 512 : (n + 1) * 512],
                    osb[:],
                )

            def make_split(mm_fn, copy_fn):
                cell = {}

                def p1(alt=False):
                    cell["ps"] = mm_fn(alt)

                def p2(alt=False):
                    copy_fn(cell["ps"])

                return p1, p2

            # ---- filler queue ----
            from collections import deque

            filler_q = deque()
            chunk_done = [0]

            def pop_filler(k):
                for _ in range(k):
                    if not filler_q:
                        return
                    n_final, fn = filler_q.popleft()
                    fn()
                    if n_final is not None:
                        chunk_done[0] = max(chunk_done[0], n_final)

            def drain_through_chunk(n):
                while filler_q and chunk_done[0] < n:
                    pop_filler(1)

            deferred = []

            def flush_norms():
                while deferred:
                    deferred.pop(0)()

            # exp routing accumulator
            route_acc = [0.0]

            def route_dve():
                route_acc[0] += DVE_SHARE
                if route_acc[0] >= 1.0:
                    route_acc[0] -= 1.0
                    return True
                return False

            # S operand tables per head: (KT_lo, KT_hi, QT_lo, QT_hi)
            SOPS = (
                (KTa, KTb, QTa, QTb),
                (KTb, KTa, QTb, QTa),
                (QKb, QKa, QKa, QKb),
            )

            gstep = [0]

            def attn_block(i, h):
                accurate = i == 0
                kt_lo, kt_hi, qt_lo, qt_hi = SOPS[h]
                yps = y_psum.tile([P, 512], F32, tag="y0")
                vsrc = VNB if accurate else VN
                clast = 2 * i + 1
                pending = []

                def emit_pv(ent):
                    pt, c, off0, off1, first, last = ent
                    if not accurate:
                        nc.tensor.matmul(
                            yps[:],
                            vsrc[:, c, h, :, :],
                            pt[:].rearrange("p (s n) -> p s n", s=2),
                            start=first,
                            stop=last,
                            perf_mode=DR,
                        )
                    else:
                        nc.tensor.matmul(
                            yps[:, off0:],
                            vsrc[:, c, h, 0, :],
                            pt[:, off0:512],
                            start=first,
                            stop=False,
                        )
                        nc.tensor.matmul(
                            yps[:, off1:],
                            vsrc[:, c, h, 1, :],
                            pt[:, 512 + off1 : 1024],
                            start=False,
                            stop=last,
                        )

                # separate the two DVE-forced diagonal steps: first and last
                if accurate:
                    order = list(range(2 * i + 2))
                else:
                    order = [2 * i] + list(range(2 * i)) + [2 * i + 1]
                nlast = len(order) - 1
                f1 = min(3, nlast)
                for ci, c in enumerate(order):
                    first = ci == 0
                    last = ci == nlast
                    j0, j1 = 2 * c, 2 * c + 1
                    off0 = max(0, j0 - 4 * i) * P
                    off1 = max(0, j1 - 4 * i) * P
                    diag = j1 >= 4 * i
                    # ready work FIRST so it sits ahead of the dependent S
                    # matmuls in the engine FIFOs
                    if ci == f1:
                        flush_norms()
                    if len(pending) > LAG:
                        emit_pv(pending.pop(0))
                    if i < 3:
                        if ci % 2 == 1:
                            pop_filler(1)
                    elif ci % 3 == 2:
                        pop_filler(1)
                    sps = s_psum.tile(
                        [P, 1024], F32, tag=f"s{gstep[0] % 3}"
                    )
                    gstep[0] += 1
                    tri0 = accurate and j0 >= 4 * i
                    tri1 = accurate and j1 >= 4 * i
                    nc.tensor.matmul(
                        sps[:, off0:512],
                        kt_lo[0:HD, j0 * P : (j0 + 1) * P],
                        qt_lo[0:HD, i * 512 + off0 : (i + 1) * 512],
                        start=True,
                        stop=not tri0,
                        tile_position=(0, 0),
                    )
                    nc.tensor.matmul(
                        sps[:, 512 + off1 : 1024],
                        kt_hi[HD:P, j1 * P : (j1 + 1) * P],
                        qt_hi[HD:P, i * 512 + off1 : (i + 1) * 512],
                        start=True,
                        stop=not tri1,
                        tile_position=(HD, 0),
                    )
                    if tri0:
                        nc.tensor.matmul(
                            sps[:, off0 : off0 + P],
                            IDM[:],
                            MTRI[:],
                            start=False,
                            stop=True,
                            skip_group_check=True,
                        )
                    if tri1:
                        nc.tensor.matmul(
                            sps[:, 512 + off1 : 512 + off1 + P],
                            IDM[:],
                            MTRI[:],
                            start=False,
                            stop=True,
                            skip_group_check=True,
                        )
                    if accurate:
                        pt = ptb_pool.tile([P, 1024], BF16, tag="ptb")
                        nc.scalar.activation(pt[:, off0:], sps[:, off0:], AF.Exp)
                    elif diag:
                        # DVE exp with mask-bias tensor: (s*A8) + MB8 -> int8
                        # bits of fp8 exp; masked/stale lanes saturate to -0.0
                        pt = pt8_pool.tile([P, 1024], FP8, tag="pt8")
                        nc.vector.scalar_tensor_tensor(
                            pt[:].bitcast(I8),
                            sps[:],
                            A8,
                            MB8[:, c % 2, :],
                            op0=ALU.mult,
                            op1=ALU.add,
                        )
                    elif not route_dve():
                        pt = pt8_pool.tile([P, 1024], FP8, tag="pt8")
                        nc.scalar.activation(pt[:], sps[:], AF.Exp)
                    else:
                        pt = pt8_pool.tile([P, 1024], FP8, tag="pt8")
                        nc.vector.tensor_scalar(
                            pt[:].bitcast(I8),
                            sps[:],
                            A8,
                            B8,
                            op0=ALU.mult,
                            op1=ALU.add,
                        )
                    pending.append((pt, c, off0, off1, first, last))
                while pending:
                    emit_pv(pending.pop(0))

                # ---- normalize: evacuate yps -> SBUF immediately (frees the
                # single y0 psum bank for the next head), then recip/bcast/mul
                # run off-path from the SBUF copy (deferred into next head) ----
                blk = slice(i * 512, (i + 1) * 512)
                yu = yu_pool.tile([P, 512], F32, tag="yu")
                nc.scalar.activation(yu[:], yps[:], AF.Copy)
                if h == 0:
                    # Y*8 at 0-63, l*8 at partition 64: hop l down to part 0
                    lr = rsb_pool.tile([1, 512], F32, tag="lr0", bufs=1)
                    rb0 = rb_pool.tile([P, 512], F32, tag="rb0", bufs=1)
                    nc.gpsimd.dma_start(lr[0:1, :], yu[HD : HD + 1, :])

                    r0 = rsb_pool.tile([1, 512], F32, tag="r0", bufs=1)

                    def _norm_h0(i=i, yu=yu, lr=lr, r0=r0, rb0=rb0, blk=blk):
                        nc.vector.reciprocal_approx_fast(r0[0:1, :], lr[0:1, :])
                        nc.gpsimd.partition_broadcast(rb0[:, :], r0[0:1, :])
                        nc.vector.tensor_mul(
                            YN01[0:HD, blk], yu[0:HD, :], rb0[0:HD, :]
                        )

                    deferred.append(_norm_h0)
                else:
                    # l*8 at partition 0 of yu
                    rtag = "r1" if h == 1 else "r2"
                    r1 = rsb_pool.tile([1, 512], F32, tag=rtag, bufs=1)
                    rb1 = rb_pool.tile([P, 512], F32, tag="rb" + rtag, bufs=1)
                    dst = YN01 if h == 1 else YN2

                    def _norm_h(yu=yu, r1=r1, rb1=rb1, dst=dst, blk=blk):
                        nc.vector.reciprocal_approx_fast(r1[0:1, :], yu[0:1, :])
                        nc.gpsimd.partition_broadcast(rb1[:, :], r1[0:1, :])
                        nc.vector.tensor_mul(
                            dst[HD:P, blk], yu[HD:P, :], rb1[HD:P, :]
                        )

                    deferred.append(_norm_h)

            # ---- prologue: block-0 qkv/v dense (alternate psum banks) ----
            for m in range(3):
                ps = qkv_mms(m, 0, alt=(m % 2 == 1))
                qkv_copy(ps, m, 0)
            for mt in range(4):
                pst = v_mms(mt, alt=(mt % 2 == 0))
                v_copy(pst, mt)

            for n in range(1, NBLK):
                for m in range(3):
                    p1, p2 = make_split(
                        (lambda alt, m=m, n=n: qkv_mms(m, n, alt)),
                        (lambda ps, m=m, n=n: qkv_copy(ps, m, n)),
                    )
                    filler_q.append((None, p1))
                    filler_q.append((None, p2))
                for s in range(4):
                    p1, p2 = make_split(
                        (lambda alt, t=4 * n + s: v_mms(t, alt)),
                        (lambda ps, t=4 * n + s: v_copy(ps, t)),
                    )
                    filler_q.append((None, p1))
                    filler_q.append((n if s == 3 else None, p2))

            def append_proj(n):
                for m in range(KC):
                    p1, p2 = make_split(
                        (lambda alt, m=m, n=n: proj_mms(m, n, alt)),
                        (lambda ps, m=m, n=n: proj_copy(ps, m, n)),
                    )
                    filler_q.append((None, p1))
                    filler_q.append((None, p2))

            # ---- main pipeline ----
            for i in range(NBLK):
                drain_through_chunk(i)
                for h in range(HPC):
                    attn_block(i, h)
                    pop_filler(1)
                    # proj for block i-1: only after h0(i)'s in-loop flush has
                    # emitted ALL of block i-1's norm writes (h2 flushes here)
                    if h == 0 and i > 0:
                        append_proj(i - 1)

            flush_norms()
            append_proj(NBLK - 1)
            alt = False
            while filler_q:
                n_final, fn = filler_q.popleft()
                fn(alt=alt)
                alt = not alt

    nc.compile()
    return nc


def _per_core_inputs(c, x, w_attn, b_attn, w_proj, xt_cache):
    b, g = divmod(c, 4)
    hs = [HPC * g + j for j in range(HPC)]

    if b not in xt_cache:
        xt_cache[b] = np.ascontiguousarray(x[b].T).astype(BF16_NP)
    xt = xt_cache[b]

    sc = 1.0 / np.sqrt(np.float32(HD))
    qc = lambda h: w_attn[:, h * HD : (h + 1) * HD] * sc
    kc = lambda h: w_attn[:, C + h * HD : C + (h + 1) * HD]
    # m-groups: [q0|q1], [k0|k1], [q2|k2]
    wqk = np.concatenate(
        [qc(hs[0]), qc(hs[1]), kc(hs[0]), kc(hs[1]), qc(hs[2]), kc(hs[2])],
        axis=1,
    ).astype(BF16_NP)
    wv = np.concatenate(
        [w_attn[:, 2 * C + h * HD : 2 * C + (h + 1) * HD] for h in hs], axis=1
    ).astype(BF16_NP)

    bq = lambda h: b_attn[h * HD : (h + 1) * HD] * sc
    bk = lambda h: b_attn[C + h * HD : C + (h + 1) * HD]
    bias_qk = np.stack(
        [
            np.concatenate([bq(hs[0]), bq(hs[1])]),
            np.concatenate([bk(hs[0]), bk(hs[1])]),
            np.concatenate([bq(hs[2]), bk(hs[2])]),
        ],
        axis=1,
    ).astype(np.float32)
    bv8 = np.broadcast_to(
        8.0
        * np.concatenate(
            [b_attn[2 * C + h * HD : 2 * C + (h + 1) * HD] for h in hs]
        ).astype(np.float32)[None, :],
        (P, HPC * HD),
    ).copy()

    # wp: slot0 = [wp_h0; wp_h1]; slot1 rows 64-127 = wp_h2
    wp0 = np.concatenate(
        [
            w_proj[hs[0] * HD : (hs[0] + 1) * HD, :],
            w_proj[hs[1] * HD : (hs[1] + 1) * HD, :],
        ]
    )
    wp1 = np.concatenate(
        [
            np.zeros((HD, C), np.float32),
            w_proj[hs[2] * HD : (hs[2] + 1) * HD, :],
        ]
    )
    wp = np.stack([wp0, wp1], axis=1).astype(BF16_NP)

    kk = np.arange(P)[:, None]
    qq = np.arange(P)[None, :]
    mtri = np.where(kk <= qq, 0.0, MASKVAL).astype(BF16_NP)
    idm = np.eye(P, dtype=BF16_NP)

    # MB8: per-element bias for the DVE fp8-bits exp on diagonal steps.
    # B8 where the score is valid, -1e6 where masked or stale.
    BIG = -1.0e6
    tri = np.where(kk <= qq, B8, BIG).astype(np.float32)  # [k, q] in-tile
    mb8 = np.empty((2, P, 1024), np.float32)
    # pattern 0: step c=2i  (j0 diag at subtile 0, j1 at subtile 1)
    mb8[0, :, :] = B8
    mb8[0, :, 0:128] = tri
    mb8[0, :, 512:640] = BIG
    mb8[0, :, 640:768] = tri
    # pattern 1: step c=2i+1 (j0 diag at subtile 2, j1 at subtile 3)
    mb8[1, :, :] = B8
    mb8[1, :, 0:256] = BIG
    mb8[1, :, 256:384] = tri
    mb8[1, :, 512:896] = BIG
    mb8[1, :, 896:1024] = tri
    mb8 = np.ascontiguousarray(mb8.transpose(1, 0, 2)).astype(BF16_NP)

    return {
        "xt": xt,
        "wqk": wqk,
        "wv": wv,
        "wp": wp,
        "bias_qk": bias_qk,
        "bv8": bv8,
        "mtri": mtri,
        "idm": idm,
        "mb8": mb8,
    }


def build_in_maps(x, w_attn, b_attn, w_proj):
    x = np.asarray(x, np.float32)
    w_attn = np.asarray(w_attn, np.float32)
    b_attn = np.asarray(b_attn, np.float32)
    w_proj = np.asarray(w_proj, np.float32)

    xt_cache = {}
    return [
        _per_core_inputs(c, x, w_attn, b_attn, w_proj, xt_cache)
        for c in range(NCORES)
    ]


def kernel(x, w_attn, b_attn, w_proj, b_proj, _return_raw=False):
    x = np.asarray(x, np.float32)
    b_proj = np.asarray(b_proj, np.float32)

    if "nc" not in _CACHE:
        _CACHE["nc"] = _build_nc()
    nc = _CACHE["nc"]

    in_maps = build_in_maps(x, w_attn, b_attn, w_proj)
    res = run_bass_kernel_spmd(nc, in_maps, list(range(NCORES)))
    outs = [r["outT"] for r in res.results]

    full = np.empty((B, T, C), np.float32)
    for b in range(B):
        acc = outs[4 * b].astype(np.float32)
        for g in range(1, 4):
            acc += outs[4 * b + g].astype(np.float32)
        full[b] = acc.T
    full += b_proj[None, None, :]
    if _return_raw:
        return full, res
    return full


# revision 34
# speedup vs baseline: 1.4100x; 1.1238x over previous
"""Causal self-attention (B=2, T=4096, C=768, H=12) on 8 trn2 NeuronCores — v3.

Sharding: core c -> batch b = c//4, head group g = c%4 (3 heads per core).

v3 changes vs v2 (319us):
  - All heads self-paired: each S step computes k-tile pair (j0,j1) of ONE
    head concurrently via tile_position (0,0)/(64,0). The partition-duplicated
    Q^T/K^T come from one SBUF->SBUF "swap halves" DMA per qkv m-group block
    (QTa=[q0|q1] natural psum copy, QTb=swap(QTa)=[q1|q0]) instead of
    duplicated weight columns -> QKV shrinks to 3 m-groups.
  - PV in fp8e4 with perf_mode=DoubleRow: one MM contracts both k-tiles of a
    step (V' [128,2,128] stationary, pt [128,2,512] moving) -> halves PV
    stream time. Diagonal steps stay split (2 plain fp8 MMs) to skip the
    invalid above-diagonal columns.
  - exp split across ScalarE and VectorE: ScalarE = ACTIVATE Exp (fp8 out);
    VectorE = single tensor_scalar op computing round(s*8*log2e + 55.55) into
    int8 = the fp8e4m3 BIT PATTERN of exp(s) (Schraudolph in fp8 bits; f32->
    int8 convert is round-to-nearest + saturating on HW, so -30000-masked
    scores land at -128 = -0.0 fp8). Blocks i=0 keep an exact bf16 path
    (ACT exp -> bf16 pt, bf16 V) because early rows have tiny L_eff; i=1 uses
    ACT fp8; i>=2 steps route by a fractional accumulator to balance engines.
  - Causal masking via PE: diagonal tiles get M_tri (upper=-30000) added in
    PSUM by an identity-weight matmul appended to the S accumulation group;
    no DVE mask multiplies remain.
  - l (softmax denom) via V' col of 8.0s: h1/h2 layouts [8|V] put l*8 at psum
    partition 0 -> direct DVE reciprocal_approx_fast from PSUM; h0 keeps
    [V|8] + partition-0 hop (DMA) + deferred norm. V scaled x8 into fp8 to
    dodge e4m3 subnormals; the 8s cancel in Y*(1/(8l))*8.
  - q/k psum->SBUF copies on ScalarE (Identity + per-partition bias), proj
    psum->SBUF on ScalarE (Copy, bf16 out); output DMA'd as bf16.
"""

import os
import sys

import numpy as np

for _p in ("/opt/trn_rl_repo", "/root/.axon_site/_ro/trn_rl_repo"):
    if os.path.isdir(_p) and _p not in sys.path:
        sys.path.insert(0, _p)

import ml_dtypes

import concourse.bacc as bacc
import concourse.bass as bass
import concourse.mybir as mybir
import concourse.tile as tile
from concourse.bass_utils import run_bass_kernel_spmd

B, T, C = 2, 4096, 768
H, HD = 12, 64
NCORES = 8
HPC = 3
P = 128
NBLK = T // 512
NKT = T // 128
NPAIR = NKT // 2
KC = C // 128

F32 = mybir.dt.float32
BF16 = mybir.dt.bfloat16
FP8 = mybir.dt.float8e4
I8 = mybir.dt.int8
BF16_NP = ml_dtypes.bfloat16
FP8_NP = ml_dtypes.float8_e4m3fn
AF = mybir.ActivationFunctionType
ALU = mybir.AluOpType
DR = mybir.MatmulPerfMode.DoubleRow

LOG2E = 1.4426950408889634
A8 = 8.0 * LOG2E
B8 = 55.55

LAG = 5          # PV trails exp by LAG steps
DVE_SHARE = 0.54  # fraction of i>=1 off-diag exp steps routed to VectorE
                  # (all diagonal steps are forced to VectorE via MB8)

MASKVAL = -30000.0

_CACHE = {}


def _build_nc():
    nc = bacc.Bacc("TRN2", target_bir_lowering=False, debug=False)

    xt_d = nc.dram_tensor("xt", [C, T], BF16, kind="ExternalInput")
    wqk_d = nc.dram_tensor("wqk", [C, 3 * P], BF16, kind="ExternalInput")
    wv_d = nc.dram_tensor("wv", [C, HPC * HD], BF16, kind="ExternalInput")
    wp_d = nc.dram_tensor("wp", [P, 2, C], BF16, kind="ExternalInput")
    bias_d = nc.dram_tensor("bias_qk", [P, 3], F32, kind="ExternalInput")
    bv_d = nc.dram_tensor("bv8", [P, HPC * HD], F32, kind="ExternalInput")
    mtri_d = nc.dram_tensor("mtri", [P, P], BF16, kind="ExternalInput")
    idm_d = nc.dram_tensor("idm", [P, P], BF16, kind="ExternalInput")
    mb8_d = nc.dram_tensor("mb8", [P, 2, 1024], BF16, kind="ExternalInput")
    out_d = nc.dram_tensor("outT", [C, T], BF16, kind="ExternalOutput")

    with tile.TileContext(nc) as tc:
        with (
            tc.tile_pool(name="store", bufs=1) as store,
            tc.tile_pool(name="consts", bufs=1) as consts,
            tc.tile_pool(name="pt8_pool", bufs=10) as pt8_pool,
            tc.tile_pool(name="ptb_pool", bufs=3) as ptb_pool,
            tc.tile_pool(name="rsb_pool", bufs=2) as rsb_pool,
            tc.tile_pool(name="yu_pool", bufs=3) as yu_pool,
            tc.tile_pool(name="rb_pool", bufs=2) as rb_pool,
            tc.tile_pool(name="osb_pool", bufs=3) as osb_pool,
            tc.tile_pool(name="s_psum", bufs=1, space="PSUM") as s_psum,
            tc.tile_pool(name="y_psum", bufs=1, space="PSUM") as y_psum,
            tc.tile_pool(name="m_psum", bufs=1, space="PSUM") as m_psum,
        ):
            # ---- persistent SBUF ----
            XT = store.tile([P, KC, T], BF16)
            WQK = store.tile([P, KC, 3 * P], BF16)
            WV = store.tile([P, KC, HPC * HD], BF16)
            WP = store.tile([P, 2, C], BF16)
            QTa = store.tile([P, T], BF16)
            QTb = store.tile([P, T], BF16)
            KTa = store.tile([P, T], BF16)
            KTb = store.tile([P, T], BF16)
            QKa = store.tile([P, T], BF16)
            QKb = store.tile([P, T], BF16)
            # V' per (pair c, head h, slice s): M-layout
            #   h0: [V(0:64) | 8@64 | 0...]  (Y at psum 0-63, l*8 at 64)
            #   h1/h2: [8@0 | 0 | V(64:128)] (l*8 at psum 0, Y at 64-127)
            VN = store.tile([P, NPAIR, HPC, 2, P], FP8)
            VNB = store.tile([P, 2, HPC, 2, P], BF16)  # pairs 0-1, bf16 for i=0
            YN01 = store.tile([P, T], BF16)  # h0 rows 0-63, h1 rows 64-127
            YN2 = store.tile([P, T], BF16)   # h2 rows 64-127

            bias_qk = consts.tile([P, 3], F32)
            bv8 = consts.tile([P, HPC * HD], F32)
            MTRI = consts.tile([P, P], BF16)
            IDM = consts.tile([P, P], BF16)
            MB8 = consts.tile([P, 2, 1024], BF16)

            # ---- input DMAs: block-0 critical path first ----
            nc.sync.dma_start(WQK[:], wqk_d.rearrange("(k p) c -> p k c", p=P))
            nc.sync.dma_start(bias_qk[:], bias_d[:])
            xt_view = xt_d.rearrange("(k p) t -> p k t", p=P)
            for k in range(KC):
                nc.sync.dma_start(XT[:, k, 0:512], xt_view[:, k, 0:512])
            nc.sync.dma_start(WV[:], wv_d.rearrange("(k p) c -> p k c", p=P))
            nc.sync.dma_start(bv8[:], bv_d[:])
            nc.sync.dma_start(MTRI[:], mtri_d[:])
            nc.sync.dma_start(IDM[:], idm_d[:])
            nc.sync.dma_start(MB8[:], mb8_d[:])
            nc.sync.dma_start(WP[:], wp_d[:])
            for n in range(1, NBLK):
                nc.gpsimd.dma_start(
                    XT[:, :, n * 512 : (n + 1) * 512],
                    xt_view[:, :, n * 512 : (n + 1) * 512],
                )

            nc.any.memset(VN[:], 0.0)
            nc.any.memset(VN[:, :, 0, :, HD : HD + 1], 8.0)
            nc.any.memset(VN[:, :, 1, :, 0:1], 8.0)
            nc.any.memset(VN[:, :, 2, :, 0:1], 8.0)
            nc.any.memset(VNB[:], 0.0)
            nc.any.memset(VNB[:, :, 0, :, HD : HD + 1], 8.0)
            nc.any.memset(VNB[:, :, 1, :, 0:1], 8.0)
            nc.any.memset(VNB[:, :, 2, :, 0:1], 8.0)

            # ---- qkv / v / proj groups ----
            DSTA = (QTa, KTa, QKa)
            DSTB = (QTb, KTb, QKb)

            def misc_tile(alt):
                if alt:
                    mt_y = y_psum.tile([P, 512], F32, tag="y0")
                    return mt_y
                mt_m = m_psum.tile([P, 512], F32, tag="misc")
                return mt_m

            # fillers are split into an MM part and a copy part so the
            # ACT/DVE copy never sits at an engine-queue head waiting on
            # its own matmul (strict per-engine FIFO would stall the exps
            # queued behind it).
            def qkv_mms(m, n, alt=False):
                ps = misc_tile(alt)
                for k in range(KC):
                    nc.tensor.matmul(
                        ps[:],
                        WQK[:, k, m * P : (m + 1) * P],
                        XT[:, k, n * 512 : (n + 1) * 512],
                        start=(k == 0),
                        stop=(k == KC - 1),
                    )
                return ps

            def qkv_copy(ps, m, n):
                blk = slice(n * 512, (n + 1) * 512)
                da, db = DSTA[m], DSTB[m]
                nc.scalar.activation(
                    da[:, blk], ps[:], AF.Identity, bias=bias_qk[:, m : m + 1]
                )
                nc.sync.dma_start(db[0:HD, blk], da[HD:P, blk])
                nc.sync.dma_start(db[HD:P, blk], da[0:HD, blk])

            def v_mms(mt, alt=False):
                pst = misc_tile(alt)
                for k in range(KC):
                    nc.tensor.matmul(
                        pst[:, 0 : HPC * HD],
                        XT[:, k, mt * P : (mt + 1) * P],
                        WV[:, k, :],
                        start=(k == 0),
                        stop=(k == KC - 1),
                    )
                return pst

            def v_copy(pst, mt):
                c, s = mt // 2, mt % 2
                vpv = pst[:, 0 : HPC * HD].rearrange("p (h d) -> p h d", h=HPC)
                bvv = bv8[:].rearrange("p (h d) -> p h d", h=HPC)
                nc.vector.scalar_tensor_tensor(
                    VN[:, c, 0, s, 0:HD], pst[:, 0:HD], 8.0, bv8[:, 0:HD],
                    op0=ALU.mult, op1=ALU.add,
                )
                nc.vector.scalar_tensor_tensor(
                    VN[:, c, 1:3, s, HD:P], vpv[:, 1:3, :], 8.0, bvv[:, 1:3, :],
                    op0=ALU.mult, op1=ALU.add,
                )
                if mt < 4:
                    nc.vector.scalar_tensor_tensor(
                        VNB[:, c, 0, s, 0:HD], pst[:, 0:HD], 8.0, bv8[:, 0:HD],
                        op0=ALU.mult, op1=ALU.add,
                    )
                    nc.vector.scalar_tensor_tensor(
                        VNB[:, c, 1:3, s, HD:P], vpv[:, 1:3, :], 8.0,
                        bvv[:, 1:3, :], op0=ALU.mult, op1=ALU.add,
                    )

            def proj_mms(m, n, alt=False):
                ops = misc_tile(alt)
                nc.tensor.matmul(
                    ops[:],
                    WP[:, 0, m * P : (m + 1) * P],
                    YN01[:, n * 512 : (n + 1) * 512],
                    start=True,
                    stop=False,
                )
                nc.tensor.matmul(
                    ops[:],
                    WP[HD:P, 1, m * P : (m + 1) * P],
                    YN2[HD:P, n * 512 : (n + 1) * 512],
                    start=False,
                    stop=True,
                    tile_position=(HD, 0),
                )
                return ops

            def proj_copy(ops, m, n):
                osb = osb_pool.tile([P, 512], BF16)
                nc.scalar.activation(osb[:], ops[:], AF.Copy)
                nc.sync.dma_start(
                    out_d[m * P : (m + 1) * P, n * 512 : (n + 1) * 512],
                    osb[:],
                )

            def make_split(mm_fn, copy_fn):
                cell = {}

                def p1(alt=False):
                    cell["ps"] = mm_fn(alt)

                def p2(alt=False):
                    copy_fn(cell["ps"])

                return p1, p2

            # ---- filler queue ----
            from collections import deque

            filler_q = deque()
            chunk_done = [0]

            def pop_filler(k):
                for _ in range(k):
                    if not filler_q:
                        return
                    n_final, fn = filler_q.popleft()
                    fn()
                    if n_final is not None:
                        chunk_done[0] = max(chunk_done[0], n_final)

            def drain_through_chunk(n):
                while filler_q and chunk_done[0] < n:
                    pop_filler(1)

            deferred = []

            def flush_norms():
                while deferred:
                    deferred.pop(0)()

            # exp routing accumulator
            route_acc = [0.0]

            def route_dve():
                route_acc[0] += DVE_SHARE
                if route_acc[0] >= 1.0:
                    route_acc[0] -= 1.0
                    return True
                return False

            # S operand tables per head: (KT_lo, KT_hi, QT_lo, QT_hi)
            SOPS = (
                (KTa, KTb, QTa, QTb),
                (KTb, KTa, QTb, QTa),
                (QKb, QKa, QKa, QKb),
            )

            gstep = [0]

            def attn_block(i, h):
                accurate = i == 0
                act_exp = accurate or i == 1
                kt_lo, kt_hi, qt_lo, qt_hi = SOPS[h]
                yps = y_psum.tile([P, 512], F32, tag="y0")
                vsrc = VNB if accurate else VN
                clast = 2 * i + 1
                pending = []

                def emit_pv(ent):
                    pt, c, off0, off1 = ent
                    if off0 == 0 and off1 == 0 and not accurate:
                        nc.tensor.matmul(
                            yps[:],
                            vsrc[:, c, h, :, :],
                            pt[:].rearrange("p (s n) -> p s n", s=2),
                            start=(c == 0),
                            stop=False,
                            perf_mode=DR,
                        )
                    else:
                        nc.tensor.matmul(
                            yps[:, off0:],
                            vsrc[:, c, h, 0, :],
                            pt[:, off0:512],
                            start=(c == 0),
                            stop=False,
                        )
                        nc.tensor.matmul(
                            yps[:, off1:],
                            vsrc[:, c, h, 1, :],
                            pt[:, 512 + off1 : 1024],
                            start=False,
                            stop=(c == clast),
                        )

                for c in range(2 * i + 2):
                    j0, j1 = 2 * c, 2 * c + 1
                    off0 = max(0, j0 - 4 * i) * P
                    off1 = max(0, j1 - 4 * i) * P
                    sps = s_psum.tile(
                        [P, 1024], F32, tag=f"s{gstep[0] % 3}"
                    )
                    gstep[0] += 1
                    tri0 = j0 >= 4 * i
                    tri1 = j1 >= 4 * i
                    nc.tensor.matmul(
                        sps[:, off0:512],
                        kt_lo[0:HD, j0 * P : (j0 + 1) * P],
                        qt_lo[0:HD, i * 512 + off0 : (i + 1) * 512],
                        start=True,
                        stop=not tri0,
                        tile_position=(0, 0),
                    )
                    nc.tensor.matmul(
                        sps[:, 512 + off1 : 1024],
                        kt_hi[HD:P, j1 * P : (j1 + 1) * P],
                        qt_hi[HD:P, i * 512 + off1 : (i + 1) * 512],
                        start=True,
                        stop=not tri1,
                        tile_position=(HD, 0),
                    )
                    if tri0:
                        nc.tensor.matmul(
                            sps[:, off0 : off0 + P],
                            IDM[:],
                            MTRI[:],
                            start=False,
                            stop=True,
                            skip_group_check=True,
                        )
                    if tri1:
                        nc.tensor.matmul(
                            sps[:, 512 + off1 : 512 + off1 + P],
                            IDM[:],
                            MTRI[:],
                            start=False,
                            stop=True,
                            skip_group_check=True,
                        )
                    if accurate:
                        pt = ptb_pool.tile([P, 1024], BF16, tag="ptb")
                        nc.scalar.activation(pt[:, off0:], sps[:, off0:], AF.Exp)
                    elif act_exp or not route_dve():
                        pt = pt8_pool.tile([P, 1024], FP8, tag="pt8")
                        nc.scalar.activation(pt[:, off0:], sps[:, off0:], AF.Exp)
                    else:
                        pt = pt8_pool.tile([P, 1024], FP8, tag="pt8")
                        nc.vector.tensor_scalar(
                            pt[:, off0:].bitcast(I8),
                            sps[:, off0:],
                            A8,
                            B8,
                            op0=ALU.mult,
                            op1=ALU.add,
                        )
                    pending.append((pt, c, off0, off1))
                    if len(pending) > LAG:
                        emit_pv(pending.pop(0))
                    if h == 1 and c == 1:
                        flush_norms()
                    if i < 3:
                        if c % 2 == 1:
                            pop_filler(1)
                    elif c % 3 == 2:
                        pop_filler(1)
                pop_filler(1)
                while pending:
                    emit_pv(pending.pop(0))

                # ---- normalize: evacuate yps -> SBUF, then recip off-path ----
                blk = slice(i * 512, (i + 1) * 512)
                yu = yu_pool.tile([P, 512], F32, tag="yu")
                nc.scalar.activation(yu[:], yps[:], AF.Copy)
                if h == 0:
                    # Y*8 at 0-63, l*8 at partition 64: hop l down to part 0
                    lr = rsb_pool.tile([1, 512], F32, tag="lr0", bufs=1)
                    r0 = rsb_pool.tile([1, 512], F32, tag="r0", bufs=1)
                    rb0 = rb_pool.tile([P, 512], F32, tag="rb0", bufs=1)
                    nc.gpsimd.dma_start(lr[0:1, :], yu[HD : HD + 1, :])

                    def _norm_h0(i=i, yu=yu, lr=lr, r0=r0, rb0=rb0, blk=blk):
                        nc.vector.reciprocal_approx_fast(r0[0:1, :], lr[0:1, :])
                        nc.gpsimd.partition_broadcast(rb0[:, :], r0[0:1, :])
                        nc.vector.tensor_mul(
                            YN01[0:HD, blk], yu[0:HD, :], rb0[0:HD, :]
                        )

                    deferred.append(_norm_h0)
                else:
                    # l*8 at partition 0 of yu
                    rtag = "r1" if h == 1 else "r2"
                    r1 = rsb_pool.tile([1, 512], F32, tag=rtag, bufs=1)
                    rb1 = rb_pool.tile([P, 512], F32, tag="rb" + rtag, bufs=1)
                    nc.vector.reciprocal_approx_fast(r1[0:1, :], yu[0:1, :])
                    nc.gpsimd.partition_broadcast(rb1[:, :], r1[0:1, :])
                    dst = YN01 if h == 1 else YN2
                    nc.vector.tensor_mul(
                        dst[HD:P, blk], yu[HD:P, :], rb1[HD:P, :]
                    )

            # ---- prologue: block-0 qkv/v dense (alternate psum banks) ----
            for m in range(3):
                ps = qkv_mms(m, 0, alt=(m % 2 == 1))
                qkv_copy(ps, m, 0)
            for mt in range(4):
                pst = v_mms(mt, alt=(mt % 2 == 0))
                v_copy(pst, mt)

            def qkv_full(m, n, alt=False):
                qkv_copy(qkv_mms(m, n, alt), m, n)

            def v_full(t, alt=False):
                v_copy(v_mms(t, alt), t)

            def proj_full(m, n, alt=False):
                proj_copy(proj_mms(m, n, alt), m, n)

            for n in range(1, NBLK):
                for m in range(3):
                    filler_q.append(
                        (None, lambda m=m, n=n, alt=False: qkv_full(m, n, alt))
                    )
                for s in range(4):
                    filler_q.append(
                        (
                            n if s == 3 else None,
                            lambda t=4 * n + s, alt=False: v_full(t, alt),
                        )
                    )

            # ---- main pipeline ----
            for i in range(NBLK):
                drain_through_chunk(i)
                for h in range(HPC):
                    attn_block(i, h)
                    pop_filler(1)
                for m in range(KC):
                    filler_q.append(
                        (None, lambda m=m, n=i, alt=False: proj_full(m, n, alt))
                    )

            flush_norms()
            alt = False
            while filler_q:
                n_final, fn = filler_q.popleft()
                fn(alt=alt)
                alt = not alt

    nc.compile()
    return nc


def _per_core_inputs(c, x, w_attn, b_attn, w_proj, xt_cache):
    b, g = divmod(c, 4)
    hs = [HPC * g + j for j in range(HPC)]

    if b not in xt_cache:
        xt_cache[b] = np.ascontiguousarray(x[b].T).astype(BF16_NP)
    xt = xt_cache[b]

    sc = 1.0 / np.sqrt(np.float32(HD))
    qc = lambda h: w_attn[:, h * HD : (h + 1) * HD] * sc
    kc = lambda h: w_attn[:, C + h * HD : C + (h + 1) * HD]
    # m-groups: [q0|q1], [k0|k1], [q2|k2]
    wqk = np.concatenate(
        [qc(hs[0]), qc(hs[1]), kc(hs[0]), kc(hs[1]), qc(hs[2]), kc(hs[2])],
        axis=1,
    ).astype(BF16_NP)
    wv = np.concatenate(
        [w_attn[:, 2 * C + h * HD : 2 * C + (h + 1) * HD] for h in hs], axis=1
    ).astype(BF16_NP)

    bq = lambda h: b_attn[h * HD : (h + 1) * HD] * sc
    bk = lambda h: b_attn[C + h * HD : C + (h + 1) * HD]
    bias_qk = np.stack(
        [
            np.concatenate([bq(hs[0]), bq(hs[1])]),
            np.concatenate([bk(hs[0]), bk(hs[1])]),
            np.concatenate([bq(hs[2]), bk(hs[2])]),
        ],
        axis=1,
    ).astype(np.float32)
    bv8 = np.broadcast_to(
        8.0
        * np.concatenate(
            [b_attn[2 * C + h * HD : 2 * C + (h + 1) * HD] for h in hs]
        ).astype(np.float32)[None, :],
        (P, HPC * HD),
    ).copy()

    # wp: slot0 = [wp_h0; wp_h1]; slot1 rows 64-127 = wp_h2
    wp0 = np.concatenate(
        [
            w_proj[hs[0] * HD : (hs[0] + 1) * HD, :],
            w_proj[hs[1] * HD : (hs[1] + 1) * HD, :],
        ]
    )
    wp1 = np.concatenate(
        [
            np.zeros((HD, C), np.float32),
            w_proj[hs[2] * HD : (hs[2] + 1) * HD, :],
        ]
    )
    wp = np.stack([wp0, wp1], axis=1).astype(BF16_NP)

    kk = np.arange(P)[:, None]
    qq = np.arange(P)[None, :]
    mtri = np.where(kk <= qq, 0.0, MASKVAL).astype(BF16_NP)
    idm = np.eye(P, dtype=BF16_NP)

    # MB8: per-element bias for the DVE fp8-bits exp on diagonal steps.
    # B8 where the score is valid, -1e6 where masked or stale.
    BIG = -1.0e6
    tri = np.where(kk <= qq, B8, BIG).astype(np.float32)  # [k, q] in-tile
    mb8 = np.empty((2, P, 1024), np.float32)
    # pattern 0: step c=2i  (j0 diag at subtile 0, j1 at subtile 1)
    mb8[0, :, :] = B8
    mb8[0, :, 0:128] = tri
    mb8[0, :, 512:640] = BIG
    mb8[0, :, 640:768] = tri
    # pattern 1: step c=2i+1 (j0 diag at subtile 2, j1 at subtile 3)
    mb8[1, :, :] = B8
    mb8[1, :, 0:256] = BIG
    mb8[1, :, 256:384] = tri
    mb8[1, :, 512:896] = BIG
    mb8[1, :, 896:1024] = tri
    mb8 = np.ascontiguousarray(mb8.transpose(1, 0, 2)).astype(BF16_NP)

    return {
        "xt": xt,
        "wqk": wqk,
        "wv": wv,
        "wp": wp,
        "bias_qk": bias_qk,
        "bv8": bv8,
        "mtri": mtri,
        "idm": idm,
        "mb8": mb8,
    }


def build_in_maps(x, w_attn, b_attn, w_proj):
    x = np.asarray(x, np.float32)
    w_attn = np.asarray(w_attn, np.float32)
    b_attn = np.asarray(b_attn, np.float32)
    w_proj = np.asarray(w_proj, np.float32)

    xt_cache = {}
    return [
        _per_core_inputs(c, x, w_attn, b_attn, w_proj, xt_cache)
        for c in range(NCORES)
    ]


def kernel(x, w_attn, b_attn, w_proj, b_proj, _return_raw=False):
    x = np.asarray(x, np.float32)
    b_proj = np.asarray(b_proj, np.float32)

    if "nc" not in _CACHE:
        _CACHE["nc"] = _build_nc()
    nc = _CACHE["nc"]

    in_maps = build_in_maps(x, w_attn, b_attn, w_proj)
    res = run_bass_kernel_spmd(nc, in_maps, list(range(NCORES)))
    outs = [r["outT"] for r in res.results]

    full = np.empty((B, T, C), np.float32)
    for b in range(B):
        acc = outs[4 * b].astype(np.float32)
        for g in range(1, 4):
            acc += outs[4 * b + g].astype(np.float32)
        full[b] = acc.T
    full += b_proj[None, None, :]
    if _return_raw:
        return full, res
    return full


# revision 36
# speedup vs baseline: 1.4398x; 1.0212x over previous
"""Causal self-attention (B=2, T=4096, C=768, H=12) on 8 trn2 NeuronCores — v3.

Sharding: core c -> batch b = c//4, head group g = c%4 (3 heads per core).

v3 changes vs v2 (319us):
  - All heads self-paired: each S step computes k-tile pair (j0,j1) of ONE
    head concurrently via tile_position (0,0)/(64,0). The partition-duplicated
    Q^T/K^T come from one SBUF->SBUF "swap halves" DMA per qkv m-group block
    (QTa=[q0|q1] natural psum copy, QTb=swap(QTa)=[q1|q0]) instead of
    duplicated weight columns -> QKV shrinks to 3 m-groups.
  - PV in fp8e4 with perf_mode=DoubleRow: one MM contracts both k-tiles of a
    step (V' [128,2,128] stationary, pt [128,2,512] moving) -> halves PV
    stream time. Diagonal steps stay split (2 plain fp8 MMs) to skip the
    invalid above-diagonal columns.
  - exp split across ScalarE and VectorE: ScalarE = ACTIVATE Exp (fp8 out);
    VectorE = single tensor_scalar op computing round(s*8*log2e + 55.55) into
    int8 = the fp8e4m3 BIT PATTERN of exp(s) (Schraudolph in fp8 bits; f32->
    int8 convert is round-to-nearest + saturating on HW, so -30000-masked
    scores land at -128 = -0.0 fp8). Blocks i=0 keep an exact bf16 path
    (ACT exp -> bf16 pt, bf16 V) because early rows have tiny L_eff; i=1 uses
    ACT fp8; i>=2 steps route by a fractional accumulator to balance engines.
  - Causal masking via PE: diagonal tiles get M_tri (upper=-30000) added in
    PSUM by an identity-weight matmul appended to the S accumulation group;
    no DVE mask multiplies remain.
  - l (softmax denom) via V' col of 8.0s: h1/h2 layouts [8|V] put l*8 at psum
    partition 0 -> direct DVE reciprocal_approx_fast from PSUM; h0 keeps
    [V|8] + partition-0 hop (DMA) + deferred norm. V scaled x8 into fp8 to
    dodge e4m3 subnormals; the 8s cancel in Y*(1/(8l))*8.
  - q/k psum->SBUF copies on ScalarE (Identity + per-partition bias), proj
    psum->SBUF on ScalarE (Copy, bf16 out); output DMA'd as bf16.
"""

import os
import sys

import numpy as np

for _p in ("/opt/trn_rl_repo", "/root/.axon_site/_ro/trn_rl_repo"):
    if os.path.isdir(_p) and _p not in sys.path:
        sys.path.insert(0, _p)

import ml_dtypes

import concourse.bacc as bacc
import concourse.bass as bass
import concourse.mybir as mybir
import concourse.tile as tile
from concourse.bass_utils import run_bass_kernel_spmd

B, T, C = 2, 4096, 768
H, HD = 12, 64
NCORES = 8
HPC = 3
P = 128
NBLK = T // 512
NKT = T // 128
NPAIR = NKT // 2
KC = C // 128

F32 = mybir.dt.float32
BF16 = mybir.dt.bfloat16
FP8 = mybir.dt.float8e4
I8 = mybir.dt.int8
BF16_NP = ml_dtypes.bfloat16
FP8_NP = ml_dtypes.float8_e4m3fn
AF = mybir.ActivationFunctionType
ALU = mybir.AluOpType
DR = mybir.MatmulPerfMode.DoubleRow

LOG2E = 1.4426950408889634
A8 = 8.0 * LOG2E
B8 = 55.55

LAG = 5          # PV trails exp by LAG steps
DVE_SHARE = 0.54  # fraction of i>=1 off-diag exp steps routed to VectorE
                  # (all diagonal steps are forced to VectorE via MB8)

MASKVAL = -30000.0

_CACHE = {}


def _build_nc():
    nc = bacc.Bacc("TRN2", target_bir_lowering=False, debug=False)

    xt_d = nc.dram_tensor("xt", [C, T], BF16, kind="ExternalInput")
    wqk_d = nc.dram_tensor("wqk", [C, 3 * P], BF16, kind="ExternalInput")
    wv_d = nc.dram_tensor("wv", [C, HPC * HD], BF16, kind="ExternalInput")
    wp_d = nc.dram_tensor("wp", [P, 2, C], BF16, kind="ExternalInput")
    bias_d = nc.dram_tensor("bias_qk", [P, 3], F32, kind="ExternalInput")
    bv_d = nc.dram_tensor("bv8", [P, HPC * HD], F32, kind="ExternalInput")
    mtri_d = nc.dram_tensor("mtri", [P, P], BF16, kind="ExternalInput")
    idm_d = nc.dram_tensor("idm", [P, P], BF16, kind="ExternalInput")
    mb8_d = nc.dram_tensor("mb8", [P, 2, 1024], BF16, kind="ExternalInput")
    out_d = nc.dram_tensor("outT", [C, T], BF16, kind="ExternalOutput")

    with tile.TileContext(nc) as tc:
        with (
            tc.tile_pool(name="store", bufs=1) as store,
            tc.tile_pool(name="consts", bufs=1) as consts,
            tc.tile_pool(name="pt8_pool", bufs=10) as pt8_pool,
            tc.tile_pool(name="ptb_pool", bufs=3) as ptb_pool,
            tc.tile_pool(name="rsb_pool", bufs=2) as rsb_pool,
            tc.tile_pool(name="yu_pool", bufs=3) as yu_pool,
            tc.tile_pool(name="rb_pool", bufs=2) as rb_pool,
            tc.tile_pool(name="osb_pool", bufs=3) as osb_pool,
            tc.tile_pool(name="s_psum", bufs=1, space="PSUM") as s_psum,
            tc.tile_pool(name="y_psum", bufs=1, space="PSUM") as y_psum,
            tc.tile_pool(name="m_psum", bufs=1, space="PSUM") as m_psum,
        ):
            # ---- persistent SBUF ----
            XT = store.tile([P, KC, T], BF16)
            WQK = store.tile([P, KC, 3 * P], BF16)
            WV = store.tile([P, KC, HPC * HD], BF16)
            WP = store.tile([P, 2, C], BF16)
            QTa = store.tile([P, T], BF16)
            QTb = store.tile([P, T], BF16)
            KTa = store.tile([P, T], BF16)
            KTb = store.tile([P, T], BF16)
            QKa = store.tile([P, T], BF16)
            QKb = store.tile([P, T], BF16)
            # V' per (pair c, head h, slice s): M-layout
            #   h0: [V(0:64) | 8@64 | 0...]  (Y at psum 0-63, l*8 at 64)
            #   h1/h2: [8@0 | 0 | V(64:128)] (l*8 at psum 0, Y at 64-127)
            VN = store.tile([P, NPAIR, HPC, 2, P], FP8)
            VNB = store.tile([P, 2, HPC, 2, P], BF16)  # pairs 0-1, bf16 for i=0
            YN01 = store.tile([P, T], BF16)  # h0 rows 0-63, h1 rows 64-127
            YN2 = store.tile([P, T], BF16)   # h2 rows 64-127

            bias_qk = consts.tile([P, 3], F32)
            bv8 = consts.tile([P, HPC * HD], F32)
            MTRI = consts.tile([P, P], BF16)
            IDM = consts.tile([P, P], BF16)
            MB8 = consts.tile([P, 2, 1024], BF16)

            # ---- input DMAs: warm-up consts, then block-0 critical path ----
            nc.sync.dma_start(MTRI[:], mtri_d[:])
            nc.sync.dma_start(IDM[:], idm_d[:])
            nc.sync.dma_start(WQK[:], wqk_d.rearrange("(k p) c -> p k c", p=P))
            nc.sync.dma_start(bias_qk[:], bias_d[:])
            xt_view = xt_d.rearrange("(k p) t -> p k t", p=P)
            for k in range(KC):
                nc.sync.dma_start(XT[:, k, 0:512], xt_view[:, k, 0:512])
            nc.sync.dma_start(WV[:], wv_d.rearrange("(k p) c -> p k c", p=P))
            nc.sync.dma_start(bv8[:], bv_d[:])
            nc.sync.dma_start(MB8[:], mb8_d[:])
            nc.sync.dma_start(WP[:], wp_d[:])
            for n in range(1, NBLK):
                nc.gpsimd.dma_start(
                    XT[:, :, n * 512 : (n + 1) * 512],
                    xt_view[:, :, n * 512 : (n + 1) * 512],
                )

            # ---- HAM warm-up: keep PE busy during the input-DMA wait so the
            # clock gate reaches 8/8 before real matmuls start ----
            wps = y_psum.tile([P, 512], F32, tag="y0")
            for w in range(48):
                nc.tensor.matmul(
                    wps[:, 0:P], IDM[:], MTRI[:], start=True, stop=True
                )

            nc.any.memset(VN[:], 0.0)
            nc.any.memset(VN[:, :, 0, :, HD : HD + 1], 8.0)
            nc.any.memset(VN[:, :, 1, :, 0:1], 8.0)
            nc.any.memset(VN[:, :, 2, :, 0:1], 8.0)
            nc.any.memset(VNB[:], 0.0)
            nc.any.memset(VNB[:, :, 0, :, HD : HD + 1], 8.0)
            nc.any.memset(VNB[:, :, 1, :, 0:1], 8.0)
            nc.any.memset(VNB[:, :, 2, :, 0:1], 8.0)

            # ---- qkv / v / proj groups ----
            DSTA = (QTa, KTa, QKa)
            DSTB = (QTb, KTb, QKb)

            def misc_tile(alt):
                if alt:
                    mt_y = y_psum.tile([P, 512], F32, tag="y0")
                    return mt_y
                mt_m = m_psum.tile([P, 512], F32, tag="misc")
                return mt_m

            # fillers are split into an MM part and a copy part so the
            # ACT/DVE copy never sits at an engine-queue head waiting on
            # its own matmul (strict per-engine FIFO would stall the exps
            # queued behind it).
            def qkv_mms(m, n, alt=False):
                ps = misc_tile(alt)
                for k in range(KC):
                    nc.tensor.matmul(
                        ps[:],
                        WQK[:, k, m * P : (m + 1) * P],
                        XT[:, k, n * 512 : (n + 1) * 512],
                        start=(k == 0),
                        stop=(k == KC - 1),
                    )
                return ps

            def qkv_copy(ps, m, n):
                blk = slice(n * 512, (n + 1) * 512)
                da, db = DSTA[m], DSTB[m]
                nc.scalar.activation(
                    da[:, blk], ps[:], AF.Identity, bias=bias_qk[:, m : m + 1]
                )
                nc.sync.dma_start(db[0:HD, blk], da[HD:P, blk])
                nc.sync.dma_start(db[HD:P, blk], da[0:HD, blk])

            def v_mms(mt, alt=False):
                pst = misc_tile(alt)
                for k in range(KC):
                    nc.tensor.matmul(
                        pst[:, 0 : HPC * HD],
                        XT[:, k, mt * P : (mt + 1) * P],
                        WV[:, k, :],
                        start=(k == 0),
                        stop=(k == KC - 1),
                    )
                return pst

            def v_copy(pst, mt):
                c, s = mt // 2, mt % 2
                vpv = pst[:, 0 : HPC * HD].rearrange("p (h d) -> p h d", h=HPC)
                bvv = bv8[:].rearrange("p (h d) -> p h d", h=HPC)
                nc.vector.scalar_tensor_tensor(
                    VN[:, c, 0, s, 0:HD], pst[:, 0:HD], 8.0, bv8[:, 0:HD],
                    op0=ALU.mult, op1=ALU.add,
                )
                nc.vector.scalar_tensor_tensor(
                    VN[:, c, 1:3, s, HD:P], vpv[:, 1:3, :], 8.0, bvv[:, 1:3, :],
                    op0=ALU.mult, op1=ALU.add,
                )
                if mt < 4:
                    nc.vector.scalar_tensor_tensor(
                        VNB[:, c, 0, s, 0:HD], pst[:, 0:HD], 8.0, bv8[:, 0:HD],
                        op0=ALU.mult, op1=ALU.add,
                    )
                    nc.vector.scalar_tensor_tensor(
                        VNB[:, c, 1:3, s, HD:P], vpv[:, 1:3, :], 8.0,
                        bvv[:, 1:3, :], op0=ALU.mult, op1=ALU.add,
                    )

            def proj_mms(m, n, alt=False):
                ops = misc_tile(alt)
                nc.tensor.matmul(
                    ops[:],
                    WP[:, 0, m * P : (m + 1) * P],
                    YN01[:, n * 512 : (n + 1) * 512],
                    start=True,
                    stop=False,
                )
                nc.tensor.matmul(
                    ops[:],
                    WP[HD:P, 1, m * P : (m + 1) * P],
                    YN2[HD:P, n * 512 : (n + 1) * 512],
                    start=False,
                    stop=True,
                    tile_position=(HD, 0),
                )
                return ops

            def proj_copy(ops, m, n):
                osb = osb_pool.tile([P, 512], BF16)
                nc.scalar.activation(osb[:], ops[:], AF.Copy)
                nc.sync.dma_start(
                    out_d[m * P : (m + 1) * P, n * 512 : (n + 1) * 512],
                    osb[:],
                )

            def make_split(mm_fn, copy_fn):
                cell = {}

                def p1(alt=False):
                    cell["ps"] = mm_fn(alt)

                def p2(alt=False):
                    copy_fn(cell["ps"])

                return p1, p2

            # ---- filler queue ----
            from collections import deque

            filler_q = deque()
            chunk_done = [0]

            def pop_filler(k):
                for _ in range(k):
                    if not filler_q:
                        return
                    n_final, fn = filler_q.popleft()
                    fn()
                    if n_final is not None:
                        chunk_done[0] = max(chunk_done[0], n_final)

            def drain_through_chunk(n):
                while filler_q and chunk_done[0] < n:
                    pop_filler(1)

            deferred = []

            def flush_norms():
                while deferred:
                    deferred.pop(0)()

            # exp routing accumulator
            route_acc = [0.0]

            def route_dve():
                route_acc[0] += DVE_SHARE
                if route_acc[0] >= 1.0:
                    route_acc[0] -= 1.0
                    return True
                return False

            # S operand tables per head: (KT_lo, KT_hi, QT_lo, QT_hi)
            SOPS = (
                (KTa, KTb, QTa, QTb),
                (KTb, KTa, QTb, QTa),
                (QKb, QKa, QKa, QKb),
            )

            gstep = [0]

            def attn_block(i, h):
                accurate = i == 0
                act_exp = accurate or i == 1
                kt_lo, kt_hi, qt_lo, qt_hi = SOPS[h]
                yps = y_psum.tile([P, 512], F32, tag="y0")
                vsrc = VNB if accurate else VN
                clast = 2 * i + 1
                pending = []

                def emit_pv(ent):
                    pt, c, off0, off1 = ent
                    if off0 == 0 and off1 == 0 and not accurate:
                        nc.tensor.matmul(
                            yps[:],
                            vsrc[:, c, h, :, :],
                            pt[:].rearrange("p (s n) -> p s n", s=2),
                            start=(c == 0),
                            stop=False,
                            perf_mode=DR,
                        )
                    else:
                        nc.tensor.matmul(
                            yps[:, off0:],
                            vsrc[:, c, h, 0, :],
                            pt[:, off0:512],
                            start=(c == 0),
                            stop=False,
                        )
                        nc.tensor.matmul(
                            yps[:, off1:],
                            vsrc[:, c, h, 1, :],
                            pt[:, 512 + off1 : 1024],
                            start=False,
                            stop=(c == clast),
                        )

                for c in range(2 * i + 2):
                    j0, j1 = 2 * c, 2 * c + 1
                    off0 = max(0, j0 - 4 * i) * P
                    off1 = max(0, j1 - 4 * i) * P
                    sps = s_psum.tile(
                        [P, 1024], F32, tag=f"s{gstep[0] % 3}"
                    )
                    gstep[0] += 1
                    tri0 = j0 >= 4 * i
                    tri1 = j1 >= 4 * i
                    nc.tensor.matmul(
                        sps[:, off0:512],
                        kt_lo[0:HD, j0 * P : (j0 + 1) * P],
                        qt_lo[0:HD, i * 512 + off0 : (i + 1) * 512],
                        start=True,
                        stop=not tri0,
                        tile_position=(0, 0),
                    )
                    nc.tensor.matmul(
                        sps[:, 512 + off1 : 1024],
                        kt_hi[HD:P, j1 * P : (j1 + 1) * P],
                        qt_hi[HD:P, i * 512 + off1 : (i + 1) * 512],
                        start=True,
                        stop=not tri1,
                        tile_position=(HD, 0),
                    )
                    if tri0:
                        nc.tensor.matmul(
                            sps[:, off0 : off0 + P],
                            IDM[:],
                            MTRI[:],
                            start=False,
                            stop=True,
                            skip_group_check=True,
                        )
                    if tri1:
                        nc.tensor.matmul(
                            sps[:, 512 + off1 : 512 + off1 + P],
                            IDM[:],
                            MTRI[:],
                            start=False,
                            stop=True,
                            skip_group_check=True,
                        )
                    if accurate:
                        pt = ptb_pool.tile([P, 1024], BF16, tag="ptb")
                        nc.scalar.activation(pt[:, off0:], sps[:, off0:], AF.Exp)
                    elif act_exp or not route_dve():
                        pt = pt8_pool.tile([P, 1024], FP8, tag="pt8")
                        nc.scalar.activation(pt[:, off0:], sps[:, off0:], AF.Exp)
                    else:
                        pt = pt8_pool.tile([P, 1024], FP8, tag="pt8")
                        nc.vector.tensor_scalar(
                            pt[:, off0:].bitcast(I8),
                            sps[:, off0:],
                            A8,
                            B8,
                            op0=ALU.mult,
                            op1=ALU.add,
                        )
                    pending.append((pt, c, off0, off1))
                    if len(pending) > LAG:
                        emit_pv(pending.pop(0))
                    if h == 1 and c == 1:
                        flush_norms()
                    if i < 3:
                        if c % 2 == 1:
                            pop_filler(1)
                    elif c % 3 == 2:
                        pop_filler(1)
                pop_filler(1)
                while pending:
                    emit_pv(pending.pop(0))

                # ---- normalize: evacuate yps -> SBUF, then recip off-path ----
                blk = slice(i * 512, (i + 1) * 512)
                yu = yu_pool.tile([P, 512], F32, tag="yu")
                nc.scalar.activation(yu[:], yps[:], AF.Copy)
                if h == 0:
                    # Y*8 at 0-63, l*8 at partition 64: hop l down to part 0
                    lr = rsb_pool.tile([1, 512], F32, tag="lr0", bufs=1)
                    r0 = rsb_pool.tile([1, 512], F32, tag="r0", bufs=1)
                    rb0 = rb_pool.tile([P, 512], F32, tag="rb0", bufs=1)
                    nc.gpsimd.dma_start(lr[0:1, :], yu[HD : HD + 1, :])

                    def _norm_h0(i=i, yu=yu, lr=lr, r0=r0, rb0=rb0, blk=blk):
                        nc.vector.reciprocal_approx_fast(r0[0:1, :], lr[0:1, :])
                        nc.gpsimd.partition_broadcast(rb0[:, :], r0[0:1, :])
                        nc.vector.tensor_mul(
                            YN01[0:HD, blk], yu[0:HD, :], rb0[0:HD, :]
                        )

                    deferred.append(_norm_h0)
                else:
                    # l*8 at partition 0 of yu
                    rtag = "r1" if h == 1 else "r2"
                    r1 = rsb_pool.tile([1, 512], F32, tag=rtag, bufs=1)
                    rb1 = rb_pool.tile([P, 512], F32, tag="rb" + rtag, bufs=1)
                    nc.vector.reciprocal_approx_fast(r1[0:1, :], yu[0:1, :])
                    nc.gpsimd.partition_broadcast(rb1[:, :], r1[0:1, :])
                    dst = YN01 if h == 1 else YN2
                    nc.vector.tensor_mul(
                        dst[HD:P, blk], yu[HD:P, :], rb1[HD:P, :]
                    )

            # ---- prologue: block-0 qkv/v dense (alternate psum banks) ----
            for m in range(3):
                ps = qkv_mms(m, 0, alt=(m % 2 == 1))
                qkv_copy(ps, m, 0)
            for mt in range(4):
                pst = v_mms(mt, alt=(mt % 2 == 0))
                v_copy(pst, mt)

            def qkv_full(m, n, alt=False):
                qkv_copy(qkv_mms(m, n, alt), m, n)

            def v_full(t, alt=False):
                v_copy(v_mms(t, alt), t)

            def proj_full(m, n, alt=False):
                proj_copy(proj_mms(m, n, alt), m, n)

            for n in range(1, NBLK):
                for m in range(3):
                    filler_q.append(
                        (None, lambda m=m, n=n, alt=False: qkv_full(m, n, alt))
                    )
                for s in range(4):
                    filler_q.append(
                        (
                            n if s == 3 else None,
                            lambda t=4 * n + s, alt=False: v_full(t, alt),
                        )
                    )

            # ---- main pipeline ----
            for i in range(NBLK):
                drain_through_chunk(i)
                for h in range(HPC):
                    attn_block(i, h)
                    pop_filler(1)
                for m in range(KC):
                    filler_q.append(
                        (None, lambda m=m, n=i, alt=False: proj_full(m, n, alt))
                    )

            flush_norms()
            alt = False
            while filler_q:
                n_final, fn = filler_q.popleft()
                fn(alt=alt)
                alt = not alt

    nc.compile()
    return nc


def _per_core_inputs(c, x, w_attn, b_attn, w_proj, xt_cache):
    b, g = divmod(c, 4)
    hs = [HPC * g + j for j in range(HPC)]

    if b not in xt_cache:
        xt_cache[b] = np.ascontiguousarray(x[b].T).astype(BF16_NP)
    xt = xt_cache[b]

    sc = 1.0 / np.sqrt(np.float32(HD))
    qc = lambda h: w_attn[:, h * HD : (h + 1) * HD] * sc
    kc = lambda h: w_attn[:, C + h * HD : C + (h + 1) * HD]
    # m-groups: [q0|q1], [k0|k1], [q2|k2]
    wqk = np.concatenate(
        [qc(hs[0]), qc(hs[1]), kc(hs[0]), kc(hs[1]), qc(hs[2]), kc(hs[2])],
        axis=1,
    ).astype(BF16_NP)
    wv = np.concatenate(
        [w_attn[:, 2 * C + h * HD : 2 * C + (h + 1) * HD] for h in hs], axis=1
    ).astype(BF16_NP)

    bq = lambda h: b_attn[h * HD : (h + 1) * HD] * sc
    bk = lambda h: b_attn[C + h * HD : C + (h + 1) * HD]
    bias_qk = np.stack(
        [
            np.concatenate([bq(hs[0]), bq(hs[1])]),
            np.concatenate([bk(hs[0]), bk(hs[1])]),
            np.concatenate([bq(hs[2]), bk(hs[2])]),
        ],
        axis=1,
    ).astype(np.float32)
    bv8 = np.broadcast_to(
        8.0
        * np.concatenate(
            [b_attn[2 * C + h * HD : 2 * C + (h + 1) * HD] for h in hs]
        ).astype(np.float32)[None, :],
        (P, HPC * HD),
    ).copy()

    # wp: slot0 = [wp_h0; wp_h1]; slot1 rows 64-127 = wp_h2
    wp0 = np.concatenate(
        [
            w_proj[hs[0] * HD : (hs[0] + 1) * HD, :],
            w_proj[hs[1] * HD : (hs[1] + 1) * HD, :],
        ]
    )
    wp1 = np.concatenate(
        [
            np.zeros((HD, C), np.float32),
            w_proj[hs[2] * HD : (hs[2] + 1) * HD, :],
        ]
    )
    wp = np.stack([wp0, wp1], axis=1).astype(BF16_NP)

    kk = np.arange(P)[:, None]
    qq = np.arange(P)[None, :]
    mtri = np.where(kk <= qq, 0.0, MASKVAL).astype(BF16_NP)
    idm = np.eye(P, dtype=BF16_NP)

    # MB8: per-element bias for the DVE fp8-bits exp on diagonal steps.
    # B8 where the score is valid, -1e6 where masked or stale.
    BIG = -1.0e6
    tri = np.where(kk <= qq, B8, BIG).astype(np.float32)  # [k, q] in-tile
    mb8 = np.empty((2, P, 1024), np.float32)
    # pattern 0: step c=2i  (j0 diag at subtile 0, j1 at subtile 1)
    mb8[0, :, :] = B8
    mb8[0, :, 0:128] = tri
    mb8[0, :, 512:640] = BIG
    mb8[0, :, 640:768] = tri
    # pattern 1: step c=2i+1 (j0 diag at subtile 2, j1 at subtile 3)
    mb8[1, :, :] = B8
    mb8[1, :, 0:256] = BIG
    mb8[1, :, 256:384] = tri
    mb8[1, :, 512:896] = BIG
    mb8[1, :, 896:1024] = tri
    mb8 = np.ascontiguousarray(mb8.transpose(1, 0, 2)).astype(BF16_NP)

    return {
        "xt": xt,
        "wqk": wqk,
        "wv": wv,
        "wp": wp,
        "bias_qk": bias_qk,
        "bv8": bv8,
        "mtri": mtri,
        "idm": idm,
        "mb8": mb8,
    }


def build_in_maps(x, w_attn, b_attn, w_proj):
    x = np.asarray(x, np.float32)
    w_attn = np.asarray(w_attn, np.float32)
    b_attn = np.asarray(b_attn, np.float32)
    w_proj = np.asarray(w_proj, np.float32)

    xt_cache = {}
    return [
        _per_core_inputs(c, x, w_attn, b_attn, w_proj, xt_cache)
        for c in range(NCORES)
    ]


def kernel(x, w_attn, b_attn, w_proj, b_proj, _return_raw=False):
    x = np.asarray(x, np.float32)
    b_proj = np.asarray(b_proj, np.float32)

    if "nc" not in _CACHE:
        _CACHE["nc"] = _build_nc()
    nc = _CACHE["nc"]

    in_maps = build_in_maps(x, w_attn, b_attn, w_proj)
    res = run_bass_kernel_spmd(nc, in_maps, list(range(NCORES)))
    outs = [r["outT"] for r in res.results]

    full = np.empty((B, T, C), np.float32)
    for b in range(B):
        acc = outs[4 * b].astype(np.float32)
        for g in range(1, 4):
            acc += outs[4 * b + g].astype(np.float32)
        full[b] = acc.T
    full += b_proj[None, None, :]
    if _return_raw:
        return full, res
    return full


# revision 37
# speedup vs baseline: 1.4446x; 1.0033x over previous
"""Causal self-attention (B=2, T=4096, C=768, H=12) on 8 trn2 NeuronCores — v3.

Sharding: core c -> batch b = c//4, head group g = c%4 (3 heads per core).

v3 changes vs v2 (319us):
  - All heads self-paired: each S step computes k-tile pair (j0,j1) of ONE
    head concurrently via tile_position (0,0)/(64,0). The partition-duplicated
    Q^T/K^T come from one SBUF->SBUF "swap halves" DMA per qkv m-group block
    (QTa=[q0|q1] natural psum copy, QTb=swap(QTa)=[q1|q0]) instead of
    duplicated weight columns -> QKV shrinks to 3 m-groups.
  - PV in fp8e4 with perf_mode=DoubleRow: one MM contracts both k-tiles of a
    step (V' [128,2,128] stationary, pt [128,2,512] moving) -> halves PV
    stream time. Diagonal steps stay split (2 plain fp8 MMs) to skip the
    invalid above-diagonal columns.
  - exp split across ScalarE and VectorE: ScalarE = ACTIVATE Exp (fp8 out);
    VectorE = single tensor_scalar op computing round(s*8*log2e + 55.55) into
    int8 = the fp8e4m3 BIT PATTERN of exp(s) (Schraudolph in fp8 bits; f32->
    int8 convert is round-to-nearest + saturating on HW, so -30000-masked
    scores land at -128 = -0.0 fp8). Blocks i=0 keep an exact bf16 path
    (ACT exp -> bf16 pt, bf16 V) because early rows have tiny L_eff; i=1 uses
    ACT fp8; i>=2 steps route by a fractional accumulator to balance engines.
  - Causal masking via PE: diagonal tiles get M_tri (upper=-30000) added in
    PSUM by an identity-weight matmul appended to the S accumulation group;
    no DVE mask multiplies remain.
  - l (softmax denom) via V' col of 8.0s: h1/h2 layouts [8|V] put l*8 at psum
    partition 0 -> direct DVE reciprocal_approx_fast from PSUM; h0 keeps
    [V|8] + partition-0 hop (DMA) + deferred norm. V scaled x8 into fp8 to
    dodge e4m3 subnormals; the 8s cancel in Y*(1/(8l))*8.
  - q/k psum->SBUF copies on ScalarE (Identity + per-partition bias), proj
    psum->SBUF on ScalarE (Copy, bf16 out); output DMA'd as bf16.
"""

import os
import sys

import numpy as np

for _p in ("/opt/trn_rl_repo", "/root/.axon_site/_ro/trn_rl_repo"):
    if os.path.isdir(_p) and _p not in sys.path:
        sys.path.insert(0, _p)

import ml_dtypes

import concourse.bacc as bacc
import concourse.bass as bass
import concourse.mybir as mybir
import concourse.tile as tile
from concourse.bass_utils import run_bass_kernel_spmd

B, T, C = 2, 4096, 768
H, HD = 12, 64
NCORES = 8
HPC = 3
P = 128
NBLK = T // 512
NKT = T // 128
NPAIR = NKT // 2
KC = C // 128

F32 = mybir.dt.float32
BF16 = mybir.dt.bfloat16
FP8 = mybir.dt.float8e4
I8 = mybir.dt.int8
BF16_NP = ml_dtypes.bfloat16
FP8_NP = ml_dtypes.float8_e4m3fn
AF = mybir.ActivationFunctionType
ALU = mybir.AluOpType
DR = mybir.MatmulPerfMode.DoubleRow

LOG2E = 1.4426950408889634
A8 = 8.0 * LOG2E
B8 = 55.55

LAG = 5          # PV trails exp by LAG steps
DVE_SHARE = 0.54  # fraction of i>=1 off-diag exp steps routed to VectorE
                  # (all diagonal steps are forced to VectorE via MB8)

MASKVAL = -30000.0

_CACHE = {}


def _build_nc():
    nc = bacc.Bacc("TRN2", target_bir_lowering=False, debug=False)

    xt_d = nc.dram_tensor("xt", [C, T], BF16, kind="ExternalInput")
    xt8_d = nc.dram_tensor("xt8", [C, T], FP8, kind="ExternalInput")
    wqk_d = nc.dram_tensor("wqk8", [C, 3 * P], FP8, kind="ExternalInput")
    wv_d = nc.dram_tensor("wv", [C, HPC * HD], BF16, kind="ExternalInput")
    wp_d = nc.dram_tensor("wp", [P, 2, C], BF16, kind="ExternalInput")
    bias_d = nc.dram_tensor("bias_qk", [P, 3], F32, kind="ExternalInput")
    bv_d = nc.dram_tensor("bv8", [P, HPC * HD], F32, kind="ExternalInput")
    mtri_d = nc.dram_tensor("mtri", [P, P], BF16, kind="ExternalInput")
    idm_d = nc.dram_tensor("idm", [P, P], BF16, kind="ExternalInput")
    mb8_d = nc.dram_tensor("mb8", [P, 2, 1024], BF16, kind="ExternalInput")
    out_d = nc.dram_tensor("outT", [C, T], BF16, kind="ExternalOutput")

    with tile.TileContext(nc) as tc:
        with (
            tc.tile_pool(name="store", bufs=1) as store,
            tc.tile_pool(name="consts", bufs=1) as consts,
            tc.tile_pool(name="pt8_pool", bufs=10) as pt8_pool,
            tc.tile_pool(name="ptb_pool", bufs=3) as ptb_pool,
            tc.tile_pool(name="rsb_pool", bufs=2) as rsb_pool,
            tc.tile_pool(name="yu_pool", bufs=3) as yu_pool,
            tc.tile_pool(name="rb_pool", bufs=2) as rb_pool,
            tc.tile_pool(name="osb_pool", bufs=3) as osb_pool,
            tc.tile_pool(name="s_psum", bufs=1, space="PSUM") as s_psum,
            tc.tile_pool(name="y_psum", bufs=1, space="PSUM") as y_psum,
            tc.tile_pool(name="m_psum", bufs=1, space="PSUM") as m_psum,
        ):
            # ---- persistent SBUF ----
            XT = store.tile([P, KC, T], BF16)
            XT8 = store.tile([P, KC // 2, 2, T], FP8)
            WQK = store.tile([P, KC // 2, 2, 3 * P], FP8)
            WV = store.tile([P, KC, HPC * HD], BF16)
            WP = store.tile([P, 2, C], BF16)
            QTa = store.tile([P, T], BF16)
            QTb = store.tile([P, T], BF16)
            KTa = store.tile([P, T], BF16)
            KTb = store.tile([P, T], BF16)
            QKa = store.tile([P, T], BF16)
            QKb = store.tile([P, T], BF16)
            # V' per (pair c, head h, slice s): M-layout
            #   h0: [V(0:64) | 8@64 | 0...]  (Y at psum 0-63, l*8 at 64)
            #   h1/h2: [8@0 | 0 | V(64:128)] (l*8 at psum 0, Y at 64-127)
            VN = store.tile([P, NPAIR, HPC, 2, P], FP8)
            VNB = store.tile([P, 2, HPC, 2, P], BF16)  # pairs 0-1, bf16 for i=0
            YN01 = store.tile([P, T], BF16)  # h0 rows 0-63, h1 rows 64-127
            YN2 = store.tile([P, T], BF16)   # h2 rows 64-127

            bias_qk = consts.tile([P, 3], F32)
            bv8 = consts.tile([P, HPC * HD], F32)
            MTRI = consts.tile([P, P], BF16)
            IDM = consts.tile([P, P], BF16)
            MB8 = consts.tile([P, 2, 1024], BF16)

            # ---- input DMAs: warm-up consts, then block-0 critical path ----
            nc.sync.dma_start(MTRI[:], mtri_d[:])
            nc.sync.dma_start(IDM[:], idm_d[:])
            nc.sync.dma_start(
                WQK[:], wqk_d.rearrange("(k s p) c -> p k s c", p=P, s=2)
            )
            nc.sync.dma_start(bias_qk[:], bias_d[:])
            xt_view = xt_d.rearrange("(k p) t -> p k t", p=P)
            xt8_view = xt8_d.rearrange("(k s p) t -> p k s t", p=P, s=2)
            nc.sync.dma_start(XT8[:, :, :, 0:512], xt8_view[:, :, :, 0:512])
            for k in range(KC):
                nc.sync.dma_start(XT[:, k, 0:512], xt_view[:, k, 0:512])
            nc.sync.dma_start(WV[:], wv_d.rearrange("(k p) c -> p k c", p=P))
            nc.sync.dma_start(bv8[:], bv_d[:])
            nc.sync.dma_start(MB8[:], mb8_d[:])
            nc.sync.dma_start(WP[:], wp_d[:])
            for n in range(1, NBLK):
                nc.gpsimd.dma_start(
                    XT[:, :, n * 512 : (n + 1) * 512],
                    xt_view[:, :, n * 512 : (n + 1) * 512],
                )
                nc.gpsimd.dma_start(
                    XT8[:, :, :, n * 512 : (n + 1) * 512],
                    xt8_view[:, :, :, n * 512 : (n + 1) * 512],
                )

            # ---- HAM warm-up: keep PE busy during the input-DMA wait so the
            # clock gate reaches 8/8 before real matmuls start ----
            wps = y_psum.tile([P, 512], F32, tag="y0")
            for w in range(48):
                nc.tensor.matmul(
                    wps[:, 0:P], IDM[:], MTRI[:], start=True, stop=True
                )

            nc.any.memset(VN[:], 0.0)
            nc.any.memset(VN[:, :, 0, :, HD : HD + 1], 8.0)
            nc.any.memset(VN[:, :, 1, :, 0:1], 8.0)
            nc.any.memset(VN[:, :, 2, :, 0:1], 8.0)
            nc.any.memset(VNB[:], 0.0)
            nc.any.memset(VNB[:, :, 0, :, HD : HD + 1], 8.0)
            nc.any.memset(VNB[:, :, 1, :, 0:1], 8.0)
            nc.any.memset(VNB[:, :, 2, :, 0:1], 8.0)

            # ---- qkv / v / proj groups ----
            DSTA = (QTa, KTa, QKa)
            DSTB = (QTb, KTb, QKb)

            def misc_tile(alt):
                if alt:
                    mt_y = y_psum.tile([P, 512], F32, tag="y0")
                    return mt_y
                mt_m = m_psum.tile([P, 512], F32, tag="misc")
                return mt_m

            # fillers are split into an MM part and a copy part so the
            # ACT/DVE copy never sits at an engine-queue head waiting on
            # its own matmul (strict per-engine FIFO would stall the exps
            # queued behind it).
            def qkv_mms(m, n, alt=False):
                ps = misc_tile(alt)
                for k in range(KC // 2):
                    nc.tensor.matmul(
                        ps[:],
                        WQK[:, k, :, m * P : (m + 1) * P],
                        XT8[:, k, :, n * 512 : (n + 1) * 512],
                        start=(k == 0),
                        stop=(k == KC // 2 - 1),
                        perf_mode=DR,
                    )
                return ps

            def qkv_copy(ps, m, n):
                blk = slice(n * 512, (n + 1) * 512)
                da, db = DSTA[m], DSTB[m]
                nc.scalar.activation(
                    da[:, blk], ps[:], AF.Identity,
                    bias=bias_qk[:, m : m + 1], scale=1.0 / 64.0,
                )
                nc.sync.dma_start(db[0:HD, blk], da[HD:P, blk])
                nc.sync.dma_start(db[HD:P, blk], da[0:HD, blk])

            def v_mms(mt, alt=False):
                pst = misc_tile(alt)
                for k in range(KC):
                    nc.tensor.matmul(
                        pst[:, 0 : HPC * HD],
                        XT[:, k, mt * P : (mt + 1) * P],
                        WV[:, k, :],
                        start=(k == 0),
                        stop=(k == KC - 1),
                    )
                return pst

            def v_copy(pst, mt):
                c, s = mt // 2, mt % 2
                vpv = pst[:, 0 : HPC * HD].rearrange("p (h d) -> p h d", h=HPC)
                bvv = bv8[:].rearrange("p (h d) -> p h d", h=HPC)
                nc.vector.scalar_tensor_tensor(
                    VN[:, c, 0, s, 0:HD], pst[:, 0:HD], 8.0, bv8[:, 0:HD],
                    op0=ALU.mult, op1=ALU.add,
                )
                nc.vector.scalar_tensor_tensor(
                    VN[:, c, 1:3, s, HD:P], vpv[:, 1:3, :], 8.0, bvv[:, 1:3, :],
                    op0=ALU.mult, op1=ALU.add,
                )
                if mt < 4:
                    nc.vector.scalar_tensor_tensor(
                        VNB[:, c, 0, s, 0:HD], pst[:, 0:HD], 8.0, bv8[:, 0:HD],
                        op0=ALU.mult, op1=ALU.add,
                    )
                    nc.vector.scalar_tensor_tensor(
                        VNB[:, c, 1:3, s, HD:P], vpv[:, 1:3, :], 8.0,
                        bvv[:, 1:3, :], op0=ALU.mult, op1=ALU.add,
                    )

            def proj_mms(m, n, alt=False):
                ops = misc_tile(alt)
                nc.tensor.matmul(
                    ops[:],
                    WP[:, 0, m * P : (m + 1) * P],
                    YN01[:, n * 512 : (n + 1) * 512],
                    start=True,
                    stop=False,
                )
                nc.tensor.matmul(
                    ops[:],
                    WP[HD:P, 1, m * P : (m + 1) * P],
                    YN2[HD:P, n * 512 : (n + 1) * 512],
                    start=False,
                    stop=True,
                    tile_position=(HD, 0),
                )
                return ops

            def proj_copy(ops, m, n):
                osb = osb_pool.tile([P, 512], BF16)
                nc.scalar.activation(osb[:], ops[:], AF.Copy)
                nc.sync.dma_start(
                    out_d[m * P : (m + 1) * P, n * 512 : (n + 1) * 512],
                    osb[:],
                )

            def make_split(mm_fn, copy_fn):
                cell = {}

                def p1(alt=False):
                    cell["ps"] = mm_fn(alt)

                def p2(alt=False):
                    copy_fn(cell["ps"])

                return p1, p2

            # ---- filler queue ----
            from collections import deque

            filler_q = deque()
            chunk_done = [0]

            def pop_filler(k):
                for _ in range(k):
                    if not filler_q:
                        return
                    n_final, fn = filler_q.popleft()
                    fn()
                    if n_final is not None:
                        chunk_done[0] = max(chunk_done[0], n_final)

            def drain_through_chunk(n):
                while filler_q and chunk_done[0] < n:
                    pop_filler(1)

            deferred = []

            def flush_norms():
                while deferred:
                    deferred.pop(0)()

            # exp routing accumulator
            route_acc = [0.0]

            def route_dve():
                route_acc[0] += DVE_SHARE
                if route_acc[0] >= 1.0:
                    route_acc[0] -= 1.0
                    return True
                return False

            # S operand tables per head: (KT_lo, KT_hi, QT_lo, QT_hi)
            SOPS = (
                (KTa, KTb, QTa, QTb),
                (KTb, KTa, QTb, QTa),
                (QKb, QKa, QKa, QKb),
            )

            gstep = [0]

            def attn_block(i, h):
                accurate = i == 0
                act_exp = accurate or i == 1
                kt_lo, kt_hi, qt_lo, qt_hi = SOPS[h]
                yps = y_psum.tile([P, 512], F32, tag="y0")
                vsrc = VNB if accurate else VN
                clast = 2 * i + 1
                pending = []

                def emit_pv(ent):
                    pt, c, off0, off1 = ent
                    if off0 == 0 and off1 == 0 and not accurate:
                        nc.tensor.matmul(
                            yps[:],
                            vsrc[:, c, h, :, :],
                            pt[:].rearrange("p (s n) -> p s n", s=2),
                            start=(c == 0),
                            stop=False,
                            perf_mode=DR,
                        )
                    else:
                        nc.tensor.matmul(
                            yps[:, off0:],
                            vsrc[:, c, h, 0, :],
                            pt[:, off0:512],
                            start=(c == 0),
                            stop=False,
                        )
                        nc.tensor.matmul(
                            yps[:, off1:],
                            vsrc[:, c, h, 1, :],
                            pt[:, 512 + off1 : 1024],
                            start=False,
                            stop=(c == clast),
                        )

                for c in range(2 * i + 2):
                    j0, j1 = 2 * c, 2 * c + 1
                    off0 = max(0, j0 - 4 * i) * P
                    off1 = max(0, j1 - 4 * i) * P
                    sps = s_psum.tile(
                        [P, 1024], F32, tag=f"s{gstep[0] % 3}"
                    )
                    gstep[0] += 1
                    tri0 = j0 >= 4 * i
                    tri1 = j1 >= 4 * i
                    nc.tensor.matmul(
                        sps[:, off0:512],
                        kt_lo[0:HD, j0 * P : (j0 + 1) * P],
                        qt_lo[0:HD, i * 512 + off0 : (i + 1) * 512],
                        start=True,
                        stop=not tri0,
                        tile_position=(0, 0),
                    )
                    nc.tensor.matmul(
                        sps[:, 512 + off1 : 1024],
                        kt_hi[HD:P, j1 * P : (j1 + 1) * P],
                        qt_hi[HD:P, i * 512 + off1 : (i + 1) * 512],
                        start=True,
                        stop=not tri1,
                        tile_position=(HD, 0),
                    )
                    if tri0:
                        nc.tensor.matmul(
                            sps[:, off0 : off0 + P],
                            IDM[:],
                            MTRI[:],
                            start=False,
                            stop=True,
                            skip_group_check=True,
                        )
                    if tri1:
                        nc.tensor.matmul(
                            sps[:, 512 + off1 : 512 + off1 + P],
                            IDM[:],
                            MTRI[:],
                            start=False,
                            stop=True,
                            skip_group_check=True,
                        )
                    if accurate:
                        pt = ptb_pool.tile([P, 1024], BF16, tag="ptb")
                        nc.scalar.activation(pt[:, off0:], sps[:, off0:], AF.Exp)
                    elif act_exp or not route_dve():
                        pt = pt8_pool.tile([P, 1024], FP8, tag="pt8")
                        nc.scalar.activation(pt[:, off0:], sps[:, off0:], AF.Exp)
                    else:
                        pt = pt8_pool.tile([P, 1024], FP8, tag="pt8")
                        nc.vector.tensor_scalar(
                            pt[:, off0:].bitcast(I8),
                            sps[:, off0:],
                            A8,
                            B8,
                            op0=ALU.mult,
                            op1=ALU.add,
                        )
                    pending.append((pt, c, off0, off1))
                    if len(pending) > LAG:
                        emit_pv(pending.pop(0))
                    if h == 1 and c == 1:
                        flush_norms()
                    if i < 3:
                        if c % 2 == 1:
                            pop_filler(1)
                    elif c % 3 == 2:
                        pop_filler(1)
                pop_filler(1)
                while pending:
                    emit_pv(pending.pop(0))

                # ---- normalize: evacuate yps -> SBUF, then recip off-path ----
                blk = slice(i * 512, (i + 1) * 512)
                yu = yu_pool.tile([P, 512], F32, tag="yu")
                nc.scalar.activation(yu[:], yps[:], AF.Copy)
                if h == 0:
                    # Y*8 at 0-63, l*8 at partition 64: hop l down to part 0
                    lr = rsb_pool.tile([1, 512], F32, tag="lr0", bufs=1)
                    r0 = rsb_pool.tile([1, 512], F32, tag="r0", bufs=1)
                    rb0 = rb_pool.tile([P, 512], F32, tag="rb0", bufs=1)
                    nc.gpsimd.dma_start(lr[0:1, :], yu[HD : HD + 1, :])

                    def _norm_h0(i=i, yu=yu, lr=lr, r0=r0, rb0=rb0, blk=blk):
                        nc.vector.reciprocal_approx_fast(r0[0:1, :], lr[0:1, :])
                        nc.gpsimd.partition_broadcast(rb0[:, :], r0[0:1, :])
                        nc.vector.tensor_mul(
                            YN01[0:HD, blk], yu[0:HD, :], rb0[0:HD, :]
                        )

                    deferred.append(_norm_h0)
                else:
                    # l*8 at partition 0 of yu
                    rtag = "r1" if h == 1 else "r2"
                    r1 = rsb_pool.tile([1, 512], F32, tag=rtag, bufs=1)
                    rb1 = rb_pool.tile([P, 512], F32, tag="rb" + rtag, bufs=1)
                    nc.vector.reciprocal_approx_fast(r1[0:1, :], yu[0:1, :])
                    nc.gpsimd.partition_broadcast(rb1[:, :], r1[0:1, :])
                    dst = YN01 if h == 1 else YN2
                    nc.vector.tensor_mul(
                        dst[HD:P, blk], yu[HD:P, :], rb1[HD:P, :]
                    )

            # ---- prologue: block-0 qkv/v dense (alternate psum banks) ----
            for m in range(3):
                ps = qkv_mms(m, 0, alt=(m % 2 == 1))
                qkv_copy(ps, m, 0)
            for mt in range(4):
                pst = v_mms(mt, alt=(mt % 2 == 0))
                v_copy(pst, mt)

            def qkv_full(m, n, alt=False):
                qkv_copy(qkv_mms(m, n, alt), m, n)

            def v_full(t, alt=False):
                v_copy(v_mms(t, alt), t)

            def proj_full(m, n, alt=False):
                proj_copy(proj_mms(m, n, alt), m, n)

            for n in range(1, NBLK):
                for m in range(3):
                    filler_q.append(
                        (None, lambda m=m, n=n, alt=False: qkv_full(m, n, alt))
                    )
                for s in range(4):
                    filler_q.append(
                        (
                            n if s == 3 else None,
                            lambda t=4 * n + s, alt=False: v_full(t, alt),
                        )
                    )

            # ---- main pipeline ----
            for i in range(NBLK):
                drain_through_chunk(i)
                for h in range(HPC):
                    attn_block(i, h)
                    pop_filler(1)
                for m in range(KC):
                    filler_q.append(
                        (None, lambda m=m, n=i, alt=False: proj_full(m, n, alt))
                    )

            flush_norms()
            alt = False
            while filler_q:
                n_final, fn = filler_q.popleft()
                fn(alt=alt)
                alt = not alt

    nc.compile()
    return nc


def _per_core_inputs(c, x, w_attn, b_attn, w_proj, xt_cache):
    b, g = divmod(c, 4)
    hs = [HPC * g + j for j in range(HPC)]

    if b not in xt_cache:
        xt_cache[b] = np.ascontiguousarray(x[b].T).astype(BF16_NP)
    xt = xt_cache[b]

    sc = 1.0 / np.sqrt(np.float32(HD))
    # x64 pre-scale lifts the tiny weights out of fp8 subnormal range; the
    # q/k copy applies 1/64. The attention 1/sqrt(HD) is folded into wq.
    qc = lambda h: w_attn[:, h * HD : (h + 1) * HD] * (sc * 64.0)
    kc = lambda h: w_attn[:, C + h * HD : C + (h + 1) * HD] * 64.0
    # m-groups: [q0|q1], [k0|k1], [q2|k2]
    wqk = np.clip(
        np.concatenate(
            [qc(hs[0]), qc(hs[1]), kc(hs[0]), kc(hs[1]), qc(hs[2]), kc(hs[2])],
            axis=1,
        ),
        -240.0,
        240.0,
    ).astype(FP8_NP)
    wv = np.concatenate(
        [w_attn[:, 2 * C + h * HD : 2 * C + (h + 1) * HD] for h in hs], axis=1
    ).astype(BF16_NP)

    bq = lambda h: b_attn[h * HD : (h + 1) * HD] * sc
    bk = lambda h: b_attn[C + h * HD : C + (h + 1) * HD]
    bias_qk = np.stack(
        [
            np.concatenate([bq(hs[0]), bq(hs[1])]),
            np.concatenate([bk(hs[0]), bk(hs[1])]),
            np.concatenate([bq(hs[2]), bk(hs[2])]),
        ],
        axis=1,
    ).astype(np.float32)
    bv8 = np.broadcast_to(
        8.0
        * np.concatenate(
            [b_attn[2 * C + h * HD : 2 * C + (h + 1) * HD] for h in hs]
        ).astype(np.float32)[None, :],
        (P, HPC * HD),
    ).copy()

    # wp: slot0 = [wp_h0; wp_h1]; slot1 rows 64-127 = wp_h2
    wp0 = np.concatenate(
        [
            w_proj[hs[0] * HD : (hs[0] + 1) * HD, :],
            w_proj[hs[1] * HD : (hs[1] + 1) * HD, :],
        ]
    )
    wp1 = np.concatenate(
        [
            np.zeros((HD, C), np.float32),
            w_proj[hs[2] * HD : (hs[2] + 1) * HD, :],
        ]
    )
    wp = np.stack([wp0, wp1], axis=1).astype(BF16_NP)

    kk = np.arange(P)[:, None]
    qq = np.arange(P)[None, :]
    mtri = np.where(kk <= qq, 0.0, MASKVAL).astype(BF16_NP)
    idm = np.eye(P, dtype=BF16_NP)

    # MB8: per-element bias for the DVE fp8-bits exp on diagonal steps.
    # B8 where the score is valid, -1e6 where masked or stale.
    BIG = -1.0e6
    tri = np.where(kk <= qq, B8, BIG).astype(np.float32)  # [k, q] in-tile
    mb8 = np.empty((2, P, 1024), np.float32)
    # pattern 0: step c=2i  (j0 diag at subtile 0, j1 at subtile 1)
    mb8[0, :, :] = B8
    mb8[0, :, 0:128] = tri
    mb8[0, :, 512:640] = BIG
    mb8[0, :, 640:768] = tri
    # pattern 1: step c=2i+1 (j0 diag at subtile 2, j1 at subtile 3)
    mb8[1, :, :] = B8
    mb8[1, :, 0:256] = BIG
    mb8[1, :, 256:384] = tri
    mb8[1, :, 512:896] = BIG
    mb8[1, :, 896:1024] = tri
    mb8 = np.ascontiguousarray(mb8.transpose(1, 0, 2)).astype(BF16_NP)

    return {
        "xt": xt,
        "xt8": np.clip(xt.astype(np.float32), -240.0, 240.0).astype(FP8_NP),
        "wqk8": wqk,
        "wv": wv,
        "wp": wp,
        "bias_qk": bias_qk,
        "bv8": bv8,
        "mtri": mtri,
        "idm": idm,
        "mb8": mb8,
    }


def build_in_maps(x, w_attn, b_attn, w_proj):
    x = np.asarray(x, np.float32)
    w_attn = np.asarray(w_attn, np.float32)
    b_attn = np.asarray(b_attn, np.float32)
    w_proj = np.asarray(w_proj, np.float32)

    xt_cache = {}
    return [
        _per_core_inputs(c, x, w_attn, b_attn, w_proj, xt_cache)
        for c in range(NCORES)
    ]


def kernel(x, w_attn, b_attn, w_proj, b_proj, _return_raw=False):
    x = np.asarray(x, np.float32)
    b_proj = np.asarray(b_proj, np.float32)

    if "nc" not in _CACHE:
        _CACHE["nc"] = _build_nc()
    nc = _CACHE["nc"]

    in_maps = build_in_maps(x, w_attn, b_attn, w_proj)
    res = run_bass_kernel_spmd(nc, in_maps, list(range(NCORES)))
    outs = [r["outT"] for r in res.results]

    full = np.empty((B, T, C), np.float32)
    for b in range(B):
        acc = outs[4 * b].astype(np.float32)
        for g in range(1, 4):
            acc += outs[4 * b + g].astype(np.float32)
        full[b] = acc.T
    full += b_proj[None, None, :]
    if _return_raw:
        return full, res
    return full
